# revision 41
# baseline (speedup 1.0000x reference)
"""SAGAN self-attention block on 8 TRN2 NeuronCores (v7, ~174-176us; v4 was 202us).

Reference (per batch element b, N = H*W = 4096, C = 512, D = 64):
    f = x @ Wf + bf ; g = x @ Wg + bg ; h = x @ Wh + bh      # [N, D]
    s = f @ g.T                                              # [N, N]
    attn = softmax(s, axis=-1)
    ctx = attn @ h                                           # [N, D]
    o = (gamma * ctx) @ Wv + bv + x                          # [N, C]

Sharding: data-parallel over batch B=8 -> one batch element per core, no
collectives. Weights replicated.

Device algorithm (per core), bf16 matmuls with f32 PSUM accumulation:
  - s is computed per m-tile (keys on partitions) in 3-m-tile groups; QK
    pairs row-pack via the FG2/GF2 stacked+mirrored f/g tensors (K=64
    streams 2 cols/cycle with tile_position).
  - softmax is unnormalized (no max subtraction); denominators ride as a
    ones-column in haug through the PV accumulation, are PE-transposed to
    per-partition scalars, and one DVE reciprocal per chunk feeds the
    fused (out*rc + x) epilogue.
  - EXP of the 16.7M logits is the ScalarE bottleneck (0.833ns/elem/lane
    = 109us floor), so the 3rd tile of each triple group is offloaded to
    the DVE as a Schraudolph exp: bf16_bits(e^s) ~= s*(2^7/ln2) + B in
    int16, one tensor_scalar per tile.  gamma=0.01 makes the attention
    term only ~0.8%% of the output norm, so the ~3%% max rel err of the
    approximation costs ~3e-5 end-to-end (gate is 2e-2).
  - PV is software-pipelined two groups behind QK/EXP so the DVE exp hop
    is never on the PE critical path; ctx PSUM ping-pongs between banks
    6/7 per chunk; epilogue (denoms at g==2, out-proj at g 3..6) rides
    inside the next chunk.
  - residual x rows and the output stream are bf16 (12.3MB DMA/core);
    DMA is spread over the two HWDGE queues (Sync: first-chunk slices,
    f/g mirrors, xr slabs; Act: weights + bulk xt slabs) plus GPSIMD
    SWDGE for output stores; final stores split 3 ways across queues.
  - chunk 0 JIT-interleaves the f/g/h projections with its QK groups;
    h accumulates in a PSUM arena in the idle odd ctx bank in rounds of
    8 chains + one CAST (the conservative tile-granular WAR between arena
    writers and CAST readers then bites once per 8 tiles, not per chain).
  - the f/g mirrors ride the GPSIMD SWDGE queue, pre-warmed by the bfg
    load (cold start ~10us), so both HWDGE queues carry xt slabs and all
    of xt lands ~7us earlier.
"""

import numpy as np
import ml_dtypes

BF16 = ml_dtypes.bfloat16

B, HH, WW, C = 8, 64, 64, 512
D = C // 8          # 64
N_FULL = HH * WW    # 4096
P = 128
CC = C // P         # 4  (c-chunks of 128)

_CACHE: dict = {}


def _groups(n_tiles):
    """m-tile groups per n-chunk: triples + a final pair (e.g. 10x3 + 1x2)."""
    gs = []
    i = 0
    while n_tiles - i >= 3:
        if n_tiles - i == 4:
            break
        gs.append([i, i + 1, i + 2])
        i += 3
    while i < n_tiles:
        gs.append(list(range(i, min(i + 2, n_tiles))))
        i += 2
    return gs


def _build(n: int, h_bias_zero: bool = False):
    import concourse.mybir as mybir
    from concourse import bacc
    from concourse.tile import TileContext

    f32 = mybir.dt.float32
    bf16 = mybir.dt.bfloat16
    i16 = mybir.dt.int16
    # Schraudolph exp in bf16-bit space: bf16_bits(exp(s)) ~= s*(2^7/ln2) + B.
    # One DVE tensor_scalar (f32 PSUM -> int16 SBUF) computes a ~3% max-rel-err
    # exp; with gamma=0.01 scaling the attention term, the end-to-end error is
    # ~3e-5.  Used for the 3rd tile of each triple group outside chunk 0 to
    # offload ~1/3 of the softmax EXP stream from the ScalarE bottleneck.
    EXP_A = float(128.0 / np.log(2.0))
    EXP_B = 16250.625
    ADD = mybir.AluOpType.add
    MULT = mybir.AluOpType.mult
    EXP = mybir.ActivationFunctionType.Exp

    n_tiles = n // P        # 32
    nch = n // 512          # 8

    nc = bacc.Bacc("TRN2", target_bir_lowering=False, debug=False)

    xr_d = nc.dram_tensor("xr", [n, C], bf16, kind="ExternalInput")
    xt_d = nc.dram_tensor("xt", [P, CC, n], bf16, kind="ExternalInput")
    wfg_d = nc.dram_tensor("wfg", [P, CC, 2 * D], bf16, kind="ExternalInput")
    wh_d = nc.dram_tensor("wh", [P, CC, D], bf16, kind="ExternalInput")
    bfg_d = nc.dram_tensor("bfg", [P, 1], f32, kind="ExternalInput")   # [bf;bg]
    if not h_bias_zero:
        bh_d = nc.dram_tensor("bhp", [1, D], bf16, kind="ExternalInput")
        on_d = nc.dram_tensor("onesp", [1, P], bf16, kind="ExternalInput")
    wv_d = nc.dram_tensor("wv", [D + 1, C], bf16, kind="ExternalInput")
    out_d = nc.dram_tensor("out", [n, C], bf16, kind="ExternalOutput")

    xr_v = xr_d.rearrange("(i p) c -> p i c", p=P)
    o_t = out_d.rearrange("(i p) c -> i p c", p=P)

    groups = _groups(n_tiles)
    need_fg = [grp[-1] // 4 for grp in groups]

    with TileContext(nc) as tc:
        with (
            tc.tile_pool(name="const", bufs=1) as cpool,
            tc.tile_pool(name="big", bufs=1) as bigpool,
            tc.tile_pool(name="ep", bufs=5) as epool,
            tc.tile_pool(name="eps", bufs=5) as epspool,
            tc.tile_pool(name="ct", bufs=2) as ctpool,
            tc.tile_pool(name="os", bufs=4) as opool,
            tc.tile_pool(name="xr", bufs=8) as xrpool,
            tc.tile_pool(name="sm", bufs=4) as smpool,
            tc.tile_pool(name="psSP", bufs=2, space="PSUM") as psSP,
            tc.tile_pool(name="psS3", bufs=2, space="PSUM") as psS3,
            tc.tile_pool(name="psE0", bufs=1, space="PSUM") as psE0,
            tc.tile_pool(name="psE1", bufs=1, space="PSUM") as psE1,
        ):
            psE = [psE0, psE1]

            # ---- replicated constants -> SBUF (small ones on qAct)
            # bfg rides the GPSIMD SWDGE queue first to warm it up: the f/g
            # mirrors use it mid-prologue and its cold start is ~10us.
            bfg_sb = cpool.tile([P, 1], f32)
            nc.gpsimd.dma_start(bfg_sb, bfg_d[:, :])
            wfg_sb = cpool.tile([P, CC, 2 * D], bf16)
            nc.scalar.dma_start(wfg_sb, wfg_d[:, :, :])

            # ---- persistent SBUF tensors
            xt = bigpool.tile([P, CC, n], bf16)          # x.T (c on partitions)
            FG2 = bigpool.tile([P, n], bf16)             # rows 0:64 f.T, 64:128 g.T
            GF2 = bigpool.tile([P, n], bf16)             # rows 0:64 g.T, 64:128 f.T
            haug = bigpool.tile([P, n_tiles, D + 1], bf16)
            nc.gpsimd.memset(haug[:, :, D:D + 1], 1.0)

            # xt DMAs: chunk 0+1 split across both HW queues (per-cc slices
            # for fast first-chunk), later chunks as one batched slab each so
            # the Sync sequencer only spends ~0.6us per chunk.
            for jc in range(2):
                for cc in range(CC):
                    sl = slice(jc * 512, (jc + 1) * 512)
                    eng = nc.sync if (cc % 2 == 0) else nc.scalar
                    eng.dma_start(xt[:, cc, sl], xt_d[:, cc, sl])

            wh_sb = cpool.tile([P, CC, D], bf16)
            nc.scalar.dma_start(wh_sb, wh_d[:, :, :])
            if not h_bias_zero:
                bh_sb = cpool.tile([1, D], bf16)
                nc.scalar.dma_start(bh_sb, bh_d[:, :])
                ones_sb = cpool.tile([1, P], bf16)
                nc.scalar.dma_start(ones_sb, on_d[:, :])
            wv_sb = cpool.tile([D + 1, C], bf16)
            nc.scalar.dma_start(wv_sb, wv_d[:, :])

            # bulk xt slabs split across BOTH HWDGE queues (mirrors ride the
            # pre-warmed SWDGE queue, so neither HW queue must stay empty);
            # all of xt lands ~7us earlier, unblocking the tail fg chunks.
            # qSyIo gets only slab-2 so the latency-critical mirror-0/1
            # transfers are never behind bulk xt in its FIFO (the SWDGE
            # queue turned out to be ~4us push-to-data + ~1.5us/transfer,
            # which made QK-g0 wait until ~24.7us)
            for jc in range(2, nch):
                sl = slice(jc * 512, (jc + 1) * 512)
                eng = nc.sync if jc == 2 else nc.scalar
                eng.dma_start(xt[:, :, sl], xt_d[:, :, sl])

            def emit_fg_chunk(jc):
                """f/g projection for 512-chunk jc -> FG2 + GF2 mirror."""
                sl = slice(jc * 512, (jc + 1) * 512)
                # fg chunks 2+ rotate through psS3, whose waits are all
                # DVE-released (schraud/bias), never ACT -- this breaks the
                # chunk-0 PE->ACT->PE stutter ring in the sp pool
                pool, tag = (psSP, "sp") if jc < 2 else (psS3, "s3")
                fgp = pool.tile([P, 512], f32, tag=tag, name=f"fg{jc}")
                for cc in range(CC):
                    nc.tensor.matmul(
                        fgp, lhsT=wfg_sb[:, cc, :], rhs=xt[:, cc, sl],
                        start=(cc == 0), stop=(cc == CC - 1),
                    )
                nc.vector.tensor_scalar(FG2[:, sl], fgp, bfg_sb, None, ADD)
                # mirror halves swapped (latency-critical: QK g0 needs them)
                nc.sync.dma_start(GF2[D:P, sl], FG2[0:D, sl])
                nc.sync.dma_start(GF2[0:D, sl], FG2[D:P, sl])

            # h arena: 8 rotating [128, 64] slots in PSUM bank 7 (psE1's bank
            # is otherwise unused until ctx_1 at chunk 1).
            hparena = psE1.tile([P, 512], f32, tag="cx", name="hparena")

            def emit_h_oct(r):
                """h projection for m-tiles 8r..8r+7: 8 accumulation chains
                into the arena, then ONE CAST into haug.  All writes precede
                the single read, so the conservative tile-granular WAR between
                arena writers and haug-CAST readers bites once per 8 tiles
                instead of once per chain."""
                for t in range(8):
                    i = 8 * r + t
                    hp = hparena[:, t * D:(t + 1) * D]
                    for cc in range(CC):
                        nc.tensor.matmul(
                            hp, lhsT=xt[:, cc, i * P:(i + 1) * P],
                            rhs=wh_sb[:, cc, :],
                            start=(cc == 0), stop=(h_bias_zero and cc == CC - 1),
                        )
                    if not h_bias_zero:
                        nc.tensor.matmul(
                            hp, lhsT=ones_sb, rhs=bh_sb, start=False, stop=True)
                # CAST on the ScalarE: it is idle until its first EXP, while
                # the DVE carries the bias->mirror chain that gates QK
                nc.scalar.copy(
                    haug[:, 8 * r:8 * r + 8, 0:D],
                    hparena.rearrange("p (t d) -> p t d", d=D))

            def emit_qk_exp(ck, g, offload=False):
                """QK + EXP for m-tile group g of column-chunk ck.
                Tiles 0-1 go to the 2-bank sp pool (released by the ScalarE
                EXP alone); tile 2 goes to its own 1-bank pool (released by
                the DVE Schraudolph alone), so a lag on either exp engine
                no longer stalls BOTH QK allocations two groups later.
                Returns (ep, ep2, ep2_is_i16)."""
                cs, cw, _ = ck
                sl = slice(cs, cs + cw)
                grp = groups[g]

                def qk(dst, q, i):
                    # row-pack QK by m-tile parity: even tiles use rows 0:64
                    # (g in GF2, f in FG2), odd tiles rows 64:128.
                    hb = (i % 2) * D
                    lhs = GF2 if hb == 0 else FG2
                    rhs = FG2 if hb == 0 else GF2
                    nc.tensor.matmul(
                        dst[:, q * 512:q * 512 + cw],
                        lhsT=lhs[hb:hb + D, i * P:(i + 1) * P],
                        rhs=rhs[hb:hb + D, sl],
                        start=True, stop=True, tile_position=(hb, 0),
                    )

                sp = psSP.tile([P, 1024], f32, tag="sp", name=f"sp{cs}_{g}")
                qk(sp, 0, grp[0])
                if len(grp) > 1:
                    qk(sp, 1, grp[1])
                sp3 = None
                if len(grp) == 3:
                    sp3 = psS3.tile([P, 512], f32, tag="s3", name=f"s3{cs}_{g}")
                    qk(sp3, 0, grp[2])
                ep = epool.tile([P, 1024], bf16, tag="ep")
                spv = sp.rearrange("p (q v) -> p q v", v=512)
                epv = ep.rearrange("p (q v) -> p q v", v=512)
                nw = min(2, len(grp))
                nc.scalar.activation(epv[:, 0:nw, 0:cw], spv[:, 0:nw, 0:cw], EXP)
                if sp3 is None:
                    return ep, None, False
                if offload:
                    epS = epspool.tile([P, 512], i16, tag="eps")
                    nc.vector.tensor_scalar(
                        epS[:, 0:cw], sp3[:, 0:cw], EXP_A, EXP_B, MULT, ADD)
                    return ep, epS, True
                ep3 = epspool.tile([P, 512], bf16, tag="eps")
                nc.scalar.activation(ep3[:, 0:cw], sp3[:, 0:cw], EXP)
                return ep, ep3, False

            def emit_pv(ck, g, ctx, eps):
                ep, ep2, is_i16 = eps
                cs, cw, _ = ck
                for q, i in enumerate(groups[g]):
                    if q == 2:
                        rhs = (ep2.bitcast(bf16) if is_i16 else ep2)[:, 0:cw]
                    else:
                        rhs = ep[:, q * 512:q * 512 + cw]
                    nc.tensor.matmul(
                        ctx[:, 0:cw], lhsT=haug[:, i, :], rhs=rhs,
                        start=(g == 0 and q == 0), stop=(i == n_tiles - 1),
                    )

            def emit_ct_copy(ck, ctx):
                """ctx -> bf16 SBUF copy (DVE)."""
                cs, cw, _ = ck
                ct = ctpool.tile([D + 1, 512], bf16, tag="ct", name=f"ct{cs}")
                nc.vector.tensor_copy(out=ct[:, 0:cw], in_=ctx[:, 0:cw])
                return ct

            def emit_denoms(ck, ct, pool):
                """PE transposes of the denominator row into one PSUM tile in
                the pending chunk's ping-pong bank, then one DVE reciprocal."""
                _, cw, tiles = ck
                # bf16 PSUM writes need 4-byte alignment: space columns 2 apart
                dt4 = pool.tile([P, 8], bf16, tag="cx", name=f"dt{tiles[0]}")
                for t in range(len(tiles)):
                    tsl = slice(t * P, (t + 1) * P)
                    nc.tensor.transpose(
                        dt4[:, 2 * t:2 * t + 1], ct[D:D + 1, tsl],
                        haug[D:D + 1, 0, D:D + 1])
                rc4 = smpool.tile([P, 8], f32, tag="rc")
                nc.vector.reciprocal(rc4, dt4)
                return rc4

            def emit_out_tile(ck, t, ct, rc, pool, tail=False):
                """out-proj + scale + residual + store for one 128-row tile."""
                it = ck[2][t]
                tsl = slice(t * P, (t + 1) * P)
                op = pool.tile([P, C], f32, tag="cx", name=f"op{it}")
                nc.tensor.matmul(op, lhsT=ct[:, tsl], rhs=wv_sb, start=True, stop=True)
                osb = opool.tile([P, C], bf16, tag="os")
                nc.vector.scalar_tensor_tensor(
                    out=osb, in0=op, scalar=rc[:, 2 * t:2 * t + 1],
                    in1=xrs_of[it // 4][:, it % 4, :], op0=MULT, op1=ADD)
                if tail:
                    # final stores: split across SWDGE + Sync + Act queues so
                    # the drain after the last compute is short
                    nc.gpsimd.dma_start(o_t[it][0:48, :], osb[0:48, :])
                    nc.sync.dma_start(o_t[it][48:96, :], osb[48:96, :])
                    nc.scalar.dma_start(o_t[it][96:P, :], osb[96:P, :])
                else:
                    nc.gpsimd.dma_start(o_t[it], osb)

            # ---- emission schedule -------------------------------------
            chunks = [(j * 512, 512, [4 * j + t for t in range(4)])
                      for j in range(nch)]

            fg_done = 0
            h_done = 0
            xrs_of = {}
            pending = None   # epilogue state: (ck, ct, pool)
            pv_q = []        # deferred PVs: (ck, g, ctx, eps), depth 2
            rcp = None

            def flush_pv(keep=2):
                # PV runs two groups behind its QK/EXP so the DVE-offloaded
                # exp tile is never on the PV critical path
                nonlocal pending
                while len(pv_q) > keep:
                    pck0, g0, ctx0, ep0 = pv_q.pop(0)
                    emit_pv(pck0, g0, ctx0, ep0)
                    if g0 == len(groups) - 1:
                        # chunk-final PV: ctx done -> bf16 copy, open epilogue
                        pending = (pck0, emit_ct_copy(pck0, ctx0),
                                   psE[(pck0[0] // 512) % 2])

            for ci, ck in enumerate(chunks):
                cs, cw, tiles = ck
                first = (ci == 0)
                if not first:
                    # residual rows (consumed by this chunk's epilogue during
                    # the next chunk); chunk 0's slab is deferred so it does
                    # not delay the latency-critical f/g mirrors on qSyIo.
                    xrc = xrpool.tile([P, 4, C], bf16, tag="xr", name=f"xr{ci}")
                    nc.gpsimd.dma_start(xrc, xr_v[:, ci * 4:(ci + 1) * 4, :])
                    xrs_of[ci] = xrc
                ctx = psE[ci % 2].tile([D + 1, 512], f32, tag="cx", name=f"ctx{cs}")
                for g, grp in enumerate(groups):
                    if first:
                        # fg rides one chunk ahead of QK demand; h pairs are
                        # emitted AFTER the group's QK/EXP so the first EXPs
                        # are never queued behind them on the in-order PE.
                        # two fg chunks per group boundary: consecutive fg
                        # allocs in the sp pool then wait each other's fast
                        # DVE bias reads instead of EXPs, and fg7's chains
                        # issue right as its xt slab lands (~19us)
                        while fg_done <= min(2 * g + 1, nch - 1):
                            emit_fg_chunk(fg_done)
                            fg_done += 1
                    ep = emit_qk_exp(ck, g, offload=not first)
                    pv_q.append((ck, g, ctx, ep))
                    flush_pv()
                    if first:
                        while (8 * h_done < 4 * fg_done
                               and 8 * h_done <= grp[-1] + 8):
                            emit_h_oct(h_done)
                            h_done += 1
                    if pending is not None:
                        pck, pct, ppool = pending
                        if g == 2:
                            rcp = emit_denoms(pck, pct, ppool)
                        elif 3 <= g <= len(pck[2]) + 2:
                            emit_out_tile(pck, g - 3, pct, rcp, ppool)
                            if g == len(pck[2]) + 2:
                                pending = None
                if first:
                    xrc = xrpool.tile([P, 4, C], bf16, tag="xr", name="xr0")
                    nc.gpsimd.dma_start(xrc, xr_v[:, 0:4, :])
                    xrs_of[0] = xrc
            flush_pv(keep=0)
            pck, pct, ppool = pending
            rcp = emit_denoms(pck, pct, ppool)
            for t in range(len(pck[2])):
                emit_out_tile(pck, t, pct, rcp, psE[t % 2], tail=True)

    nc.compile()
    return nc


def get_program(n: int = N_FULL, h_bias_zero: bool = False):
    key = (n, h_bias_zero)
    if key not in _CACHE:
        _CACHE[key] = _build(n, h_bias_zero)
    return _CACHE[key]


def make_weight_maps(Wf, bf, Wg, bg, Wh, bh, Wv, bv, gamma, h_bias_zero=False):
    """Host-side layout prep of the tiny replicated weights."""
    wv_aug = np.concatenate(
        [np.float32(gamma) * np.asarray(Wv, np.float32),
         np.asarray(bv, np.float32)[None, :]], axis=0)
    bfg = np.concatenate(
        [np.asarray(bf, np.float32), np.asarray(bg, np.float32)]).reshape(P, 1)
    wfg = np.concatenate(
        [np.asarray(Wf, np.float32), np.asarray(Wg, np.float32)], axis=1)
    # c index decomposition: c = cc*128 + p  ->  [p, cc, d]
    maps = {
        "wfg": np.ascontiguousarray(
            wfg.astype(BF16).reshape(CC, P, 2 * D).transpose(1, 0, 2)),
        "wh": np.ascontiguousarray(
            np.asarray(Wh, np.float32).astype(BF16).reshape(CC, P, D).transpose(1, 0, 2)),
        "bfg": np.ascontiguousarray(bfg),
        "bhp": np.ascontiguousarray(
            np.asarray(bh, np.float32).astype(BF16).reshape(1, D)),
        "onesp": np.ones((1, P), dtype=BF16),
        "wv": np.ascontiguousarray(wv_aug.astype(BF16)),
    }
    if h_bias_zero:
        del maps["bhp"], maps["onesp"]
    return maps


def make_x_maps(xf_b):
    """Per-core x layouts: residual rows (bf16) + transposed bf16 [p, cc, n]."""
    x = np.ascontiguousarray(xf_b, dtype=np.float32)
    xt = x.T.astype(BF16).reshape(CC, P, x.shape[0]).transpose(1, 0, 2)
    return {"xr": x.astype(BF16), "xt": np.ascontiguousarray(xt)}


def kernel(x, Wf, bf, Wg, bg, Wh, bh, Wv, bv, gamma):
    from concourse.bass_utils import run_bass_kernel_spmd

    x = np.asarray(x, np.float32)
    b, hh, ww, c = x.shape
    n = hh * ww
    assert (b, c) == (B, C)

    hbz = bool(np.all(np.asarray(bh) == 0))
    nc = get_program(n, hbz)
    base = make_weight_maps(Wf, bf, Wg, bg, Wh, bh, Wv, bv, gamma, hbz)
    xf = x.reshape(b, n, c)
    in_maps = [dict(base, **make_x_maps(xf[i])) for i in range(b)]

    res = run_bass_kernel_spmd(nc, in_maps, core_ids=list(range(b)))
    out = np.stack([np.asarray(res.results[i]["out"], np.float32)
                    for i in range(b)], axis=0)
    return np.ascontiguousarray(out.reshape(b, hh, ww, c).astype(np.float32))


# revision 43
# speedup vs baseline: 1.0417x; 1.0417x over previous
"""SAGAN self-attention block on 8 TRN2 NeuronCores (v7, ~174-176us; v4 was 202us).

Reference (per batch element b, N = H*W = 4096, C = 512, D = 64):
    f = x @ Wf + bf ; g = x @ Wg + bg ; h = x @ Wh + bh      # [N, D]
    s = f @ g.T                                              # [N, N]
    attn = softmax(s, axis=-1)
    ctx = attn @ h                                           # [N, D]
    o = (gamma * ctx) @ Wv + bv + x                          # [N, C]

Sharding: data-parallel over batch B=8 -> one batch element per core, no
collectives. Weights replicated.

Device algorithm (per core), bf16 matmuls with f32 PSUM accumulation:
  - s is computed per m-tile (keys on partitions) in 3-m-tile groups; QK
    pairs row-pack via the FG2/GF2 stacked+mirrored f/g tensors (K=64
    streams 2 cols/cycle with tile_position).
  - softmax is unnormalized (no max subtraction); denominators ride as a
    ones-column in haug through the PV accumulation, are PE-transposed to
    per-partition scalars, and one DVE reciprocal per chunk feeds the
    fused (out*rc + x) epilogue.
  - EXP of the 16.7M logits is the ScalarE bottleneck (0.833ns/elem/lane
    = 109us floor), so the 3rd tile of each triple group is offloaded to
    the DVE as a Schraudolph exp: bf16_bits(e^s) ~= s*(2^7/ln2) + B in
    int16, one tensor_scalar per tile.  gamma=0.01 makes the attention
    term only ~0.8%% of the output norm, so the ~3%% max rel err of the
    approximation costs ~3e-5 end-to-end (gate is 2e-2).
  - PV is software-pipelined two groups behind QK/EXP so the DVE exp hop
    is never on the PE critical path; ctx PSUM ping-pongs between banks
    6/7 per chunk; epilogue (denoms at g==2, out-proj at g 3..6) rides
    inside the next chunk.
  - residual x rows and the output stream are bf16 (12.3MB DMA/core);
    DMA is spread over the two HWDGE queues (Sync: first-chunk slices,
    f/g mirrors, xr slabs; Act: weights + bulk xt slabs) plus GPSIMD
    SWDGE for output stores; final stores split 3 ways across queues.
  - chunk 0 JIT-interleaves the f/g/h projections with its QK groups;
    h accumulates in a PSUM arena in the idle odd ctx bank in rounds of
    8 chains + one CAST (the conservative tile-granular WAR between arena
    writers and CAST readers then bites once per 8 tiles, not per chain).
  - the f/g mirrors ride the GPSIMD SWDGE queue, pre-warmed by the bfg
    load (cold start ~10us), so both HWDGE queues carry xt slabs and all
    of xt lands ~7us earlier.
"""

import numpy as np
import ml_dtypes

BF16 = ml_dtypes.bfloat16

B, HH, WW, C = 8, 64, 64, 512
D = C // 8          # 64
N_FULL = HH * WW    # 4096
P = 128
CC = C // P         # 4  (c-chunks of 128)

_CACHE: dict = {}


def _groups(n_tiles):
    """m-tile groups per n-chunk: triples + a final pair (e.g. 10x3 + 1x2)."""
    gs = []
    i = 0
    while n_tiles - i >= 3:
        if n_tiles - i == 4:
            break
        gs.append([i, i + 1, i + 2])
        i += 3
    while i < n_tiles:
        gs.append(list(range(i, min(i + 2, n_tiles))))
        i += 2
    return gs


def _build(n: int, h_bias_zero: bool = False):
    import concourse.mybir as mybir
    from concourse import bacc
    from concourse.tile import TileContext

    f32 = mybir.dt.float32
    bf16 = mybir.dt.bfloat16
    i16 = mybir.dt.int16
    # Schraudolph exp in bf16-bit space: bf16_bits(exp(s)) ~= s*(2^7/ln2) + B.
    # One DVE tensor_scalar (f32 PSUM -> int16 SBUF) computes a ~3% max-rel-err
    # exp; with gamma=0.01 scaling the attention term, the end-to-end error is
    # ~3e-5.  Used for the 3rd tile of each triple group outside chunk 0 to
    # offload ~1/3 of the softmax EXP stream from the ScalarE bottleneck.
    EXP_A = float(128.0 / np.log(2.0))
    EXP_B = 16250.625
    ADD = mybir.AluOpType.add
    MULT = mybir.AluOpType.mult
    EXP = mybir.ActivationFunctionType.Exp

    n_tiles = n // P        # 32
    nch = n // 512          # 8

    nc = bacc.Bacc("TRN2", target_bir_lowering=False, debug=False)

    xr_d = nc.dram_tensor("xr", [n, C], bf16, kind="ExternalInput")
    xt_d = nc.dram_tensor("xt", [P, CC, n], bf16, kind="ExternalInput")
    wfg_d = nc.dram_tensor("wfg", [P, CC, 2 * D], bf16, kind="ExternalInput")
    wh_d = nc.dram_tensor("wh", [P, CC, D], bf16, kind="ExternalInput")
    bfg_d = nc.dram_tensor("bfg", [P, 1], f32, kind="ExternalInput")   # [bf;bg]
    if not h_bias_zero:
        bh_d = nc.dram_tensor("bhp", [1, D], bf16, kind="ExternalInput")
        on_d = nc.dram_tensor("onesp", [1, P], bf16, kind="ExternalInput")
    wv_d = nc.dram_tensor("wv", [D + 1, C], bf16, kind="ExternalInput")
    out_d = nc.dram_tensor("out", [n, C], bf16, kind="ExternalOutput")

    xr_v = xr_d.rearrange("(i p) c -> p i c", p=P)
    o_t = out_d.rearrange("(i p) c -> i p c", p=P)

    groups = _groups(n_tiles)
    need_fg = [grp[-1] // 4 for grp in groups]

    with TileContext(nc) as tc:
        with (
            tc.tile_pool(name="const", bufs=1) as cpool,
            tc.tile_pool(name="big", bufs=1) as bigpool,
            tc.tile_pool(name="ep", bufs=5) as epool,
            tc.tile_pool(name="eps", bufs=5) as epspool,
            tc.tile_pool(name="ct", bufs=2) as ctpool,
            tc.tile_pool(name="os", bufs=4) as opool,
            tc.tile_pool(name="xr", bufs=8) as xrpool,
            tc.tile_pool(name="sm", bufs=4) as smpool,
            tc.tile_pool(name="psSP", bufs=2, space="PSUM") as psSP,
            tc.tile_pool(name="psS3", bufs=2, space="PSUM") as psS3,
            tc.tile_pool(name="psE0", bufs=1, space="PSUM") as psE0,
            tc.tile_pool(name="psE1", bufs=1, space="PSUM") as psE1,
        ):
            psE = [psE0, psE1]

            # ---- replicated constants -> SBUF (small ones on qAct)
            # bfg rides the GPSIMD SWDGE queue first to warm it up: the f/g
            # mirrors use it mid-prologue and its cold start is ~10us.
            bfg_sb = cpool.tile([P, 1], f32)
            nc.gpsimd.dma_start(bfg_sb, bfg_d[:, :])
            wfg_sb = cpool.tile([P, CC, 2 * D], bf16)
            nc.scalar.dma_start(wfg_sb, wfg_d[:, :, :])

            # ---- persistent SBUF tensors
            xt = bigpool.tile([P, CC, n], bf16)          # x.T (c on partitions)
            FG2 = bigpool.tile([P, n], bf16)             # rows 0:64 f.T, 64:128 g.T
            GF2 = bigpool.tile([P, n], bf16)             # rows 0:64 g.T, 64:128 f.T
            haug = bigpool.tile([P, n_tiles, D + 1], bf16)
            nc.gpsimd.memset(haug[:, :, D:D + 1], 1.0)

            # xt DMAs: chunk 0+1 split across both HW queues (per-cc slices
            # for fast first-chunk), later chunks as one batched slab each so
            # the Sync sequencer only spends ~0.6us per chunk.
            for jc in range(2):
                for cc in range(CC):
                    sl = slice(jc * 512, (jc + 1) * 512)
                    eng = nc.sync if (cc % 2 == 0) else nc.scalar
                    eng.dma_start(xt[:, cc, sl], xt_d[:, cc, sl])

            wh_sb = cpool.tile([P, CC, D], bf16)
            nc.scalar.dma_start(wh_sb, wh_d[:, :, :])
            if not h_bias_zero:
                bh_sb = cpool.tile([1, D], bf16)
                nc.scalar.dma_start(bh_sb, bh_d[:, :])
                ones_sb = cpool.tile([1, P], bf16)
                nc.scalar.dma_start(ones_sb, on_d[:, :])
            wv_sb = cpool.tile([D + 1, C], bf16)
            nc.scalar.dma_start(wv_sb, wv_d[:, :])

            # bulk xt slabs split across BOTH HWDGE queues (mirrors ride the
            # pre-warmed SWDGE queue, so neither HW queue must stay empty);
            # all of xt lands ~7us earlier, unblocking the tail fg chunks.
            # qSyIo gets only slab-2 so the latency-critical mirror-0/1
            # transfers are never behind bulk xt in its FIFO (the SWDGE
            # queue turned out to be ~4us push-to-data + ~1.5us/transfer,
            # which made QK-g0 wait until ~24.7us)
            for jc in range(2, nch):
                sl = slice(jc * 512, (jc + 1) * 512)
                eng = nc.sync if jc == 2 else nc.scalar
                eng.dma_start(xt[:, :, sl], xt_d[:, :, sl])

            def emit_fg_chunk(jc):
                """f/g projection for 512-chunk jc -> FG2 + GF2 mirror."""
                sl = slice(jc * 512, (jc + 1) * 512)
                # fg chunks 2+ rotate through psS3, whose waits are all
                # DVE-released (schraud/bias), never ACT -- this breaks the
                # chunk-0 PE->ACT->PE stutter ring in the sp pool
                pool, tag = (psSP, "sp") if jc < 2 else (psS3, "s3")
                fgp = pool.tile([P, 512], f32, tag=tag, name=f"fg{jc}")
                for cc in range(CC):
                    nc.tensor.matmul(
                        fgp, lhsT=wfg_sb[:, cc, :], rhs=xt[:, cc, sl],
                        start=(cc == 0), stop=(cc == CC - 1),
                    )
                nc.vector.tensor_scalar(FG2[:, sl], fgp, bfg_sb, None, ADD)
                # mirror halves swapped (latency-critical: QK g0 needs them)
                nc.sync.dma_start(GF2[D:P, sl], FG2[0:D, sl])
                nc.sync.dma_start(GF2[0:D, sl], FG2[D:P, sl])

            # h arena: 8 rotating [128, 64] slots in PSUM bank 7 (psE1's bank
            # is otherwise unused until ctx_1 at chunk 1).
            hparena = psE1.tile([P, 512], f32, tag="cx", name="hparena")

            def emit_h_oct(r):
                """h projection for m-tiles 8r..8r+7: 8 accumulation chains
                into the arena, then ONE CAST into haug.  All writes precede
                the single read, so the conservative tile-granular WAR between
                arena writers and haug-CAST readers bites once per 8 tiles
                instead of once per chain."""
                for t in range(8):
                    i = 8 * r + t
                    hp = hparena[:, t * D:(t + 1) * D]
                    for cc in range(CC):
                        nc.tensor.matmul(
                            hp, lhsT=xt[:, cc, i * P:(i + 1) * P],
                            rhs=wh_sb[:, cc, :],
                            start=(cc == 0), stop=(h_bias_zero and cc == CC - 1),
                        )
                    if not h_bias_zero:
                        nc.tensor.matmul(
                            hp, lhsT=ones_sb, rhs=bh_sb, start=False, stop=True)
                # CAST on the ScalarE: it is idle until its first EXP, while
                # the DVE carries the bias->mirror chain that gates QK
                nc.scalar.copy(
                    haug[:, 8 * r:8 * r + 8, 0:D],
                    hparena.rearrange("p (t d) -> p t d", d=D))

            def emit_qk_exp(ck, g, offload=False):
                """QK + EXP for m-tile group g of column-chunk ck.
                Tiles 0-1 go to the 2-bank sp pool (released by the ScalarE
                EXP alone); tile 2 goes to its own 1-bank pool (released by
                the DVE Schraudolph alone), so a lag on either exp engine
                no longer stalls BOTH QK allocations two groups later.
                Returns (ep, ep2, ep2_is_i16)."""
                cs, cw, _ = ck
                sl = slice(cs, cs + cw)
                grp = groups[g]

                def qk(dst, q, i):
                    # row-pack QK by m-tile parity: even tiles use rows 0:64
                    # (g in GF2, f in FG2), odd tiles rows 64:128.
                    hb = (i % 2) * D
                    lhs = GF2 if hb == 0 else FG2
                    rhs = FG2 if hb == 0 else GF2
                    nc.tensor.matmul(
                        dst[:, q * 512:q * 512 + cw],
                        lhsT=lhs[hb:hb + D, i * P:(i + 1) * P],
                        rhs=rhs[hb:hb + D, sl],
                        start=True, stop=True, tile_position=(hb, 0),
                    )

                sp = psSP.tile([P, 1024], f32, tag="sp", name=f"sp{cs}_{g}")
                qk(sp, 0, grp[0])
                if len(grp) > 1:
                    qk(sp, 1, grp[1])
                sp3 = None
                if len(grp) == 3:
                    sp3 = psS3.tile([P, 512], f32, tag="s3", name=f"s3{cs}_{g}")
                    qk(sp3, 0, grp[2])
                ep = epool.tile([P, 1024], bf16, tag="ep")
                spv = sp.rearrange("p (q v) -> p q v", v=512)
                epv = ep.rearrange("p (q v) -> p q v", v=512)
                nw = min(2, len(grp))
                nc.scalar.activation(epv[:, 0:nw, 0:cw], spv[:, 0:nw, 0:cw], EXP)
                if sp3 is None:
                    return ep, None, False
                if offload:
                    epS = epspool.tile([P, 512], i16, tag="eps")
                    nc.vector.tensor_scalar(
                        epS[:, 0:cw], sp3[:, 0:cw], EXP_A, EXP_B, MULT, ADD)
                    return ep, epS, True
                ep3 = epspool.tile([P, 512], bf16, tag="eps")
                nc.scalar.activation(ep3[:, 0:cw], sp3[:, 0:cw], EXP)
                return ep, ep3, False

            def emit_pv(ck, g, ctx, eps):
                ep, ep2, is_i16 = eps
                cs, cw, _ = ck
                for q, i in enumerate(groups[g]):
                    if q == 2:
                        rhs = (ep2.bitcast(bf16) if is_i16 else ep2)[:, 0:cw]
                    else:
                        rhs = ep[:, q * 512:q * 512 + cw]
                    nc.tensor.matmul(
                        ctx[:, 0:cw], lhsT=haug[:, i, :], rhs=rhs,
                        start=(g == 0 and q == 0), stop=(i == n_tiles - 1),
                    )

            def emit_ct_copy(ck, ctx):
                """ctx -> bf16 SBUF copy (DVE)."""
                cs, cw, _ = ck
                ct = ctpool.tile([D + 1, 512], bf16, tag="ct", name=f"ct{cs}")
                nc.vector.tensor_copy(out=ct[:, 0:cw], in_=ctx[:, 0:cw])
                return ct

            def emit_denoms(ck, ct, pool):
                """PE transposes of the denominator row into one PSUM tile in
                the pending chunk's ping-pong bank, then one DVE reciprocal."""
                _, cw, tiles = ck
                # bf16 PSUM writes need 4-byte alignment: space columns 2 apart
                dt4 = pool.tile([P, 8], bf16, tag="cx", name=f"dt{tiles[0]}")
                for t in range(len(tiles)):
                    tsl = slice(t * P, (t + 1) * P)
                    nc.tensor.transpose(
                        dt4[:, 2 * t:2 * t + 1], ct[D:D + 1, tsl],
                        haug[D:D + 1, 0, D:D + 1])
                rc4 = smpool.tile([P, 8], f32, tag="rc")
                nc.vector.reciprocal(rc4, dt4)
                return rc4

            def emit_out_tile(ck, t, ct, rc, pool, tail=False):
                """out-proj + scale + residual + store for one 128-row tile."""
                it = ck[2][t]
                tsl = slice(t * P, (t + 1) * P)
                op = pool.tile([P, C], f32, tag="cx", name=f"op{it}")
                nc.tensor.matmul(op, lhsT=ct[:, tsl], rhs=wv_sb, start=True, stop=True)
                osb = opool.tile([P, C], bf16, tag="os")
                nc.vector.scalar_tensor_tensor(
                    out=osb, in0=op, scalar=rc[:, 2 * t:2 * t + 1],
                    in1=xrs_of[it // 4][:, it % 4, :], op0=MULT, op1=ADD)
                if tail:
                    # final stores: split across SWDGE + Sync + Act queues so
                    # the drain after the last compute is short
                    nc.gpsimd.dma_start(o_t[it][0:48, :], osb[0:48, :])
                    nc.sync.dma_start(o_t[it][48:96, :], osb[48:96, :])
                    nc.scalar.dma_start(o_t[it][96:P, :], osb[96:P, :])
                else:
                    nc.gpsimd.dma_start(o_t[it], osb)

            # ---- emission schedule -------------------------------------
            chunks = [(j * 512, 512, [4 * j + t for t in range(4)])
                      for j in range(nch)]

            fg_done = 0
            h_done = 0
            xrs_of = {}
            pending = None   # epilogue state: (ck, ct, pool)
            pv_q = []        # deferred PVs: (ck, g, ctx, eps), depth 2
            rcp = None

            def flush_pv(keep=2):
                # PV runs two groups behind its QK/EXP so the DVE-offloaded
                # exp tile is never on the PV critical path
                nonlocal pending
                while len(pv_q) > keep:
                    pck0, g0, ctx0, ep0 = pv_q.pop(0)
                    emit_pv(pck0, g0, ctx0, ep0)
                    if g0 == len(groups) - 1:
                        # chunk-final PV: ctx done -> bf16 copy, open epilogue
                        pending = (pck0, emit_ct_copy(pck0, ctx0),
                                   psE[(pck0[0] // 512) % 2])

            for ci, ck in enumerate(chunks):
                cs, cw, tiles = ck
                first = (ci == 0)
                if not first:
                    # residual rows (consumed by this chunk's epilogue during
                    # the next chunk); chunk 0's slab is deferred so it does
                    # not delay the latency-critical f/g mirrors on qSyIo.
                    xrc = xrpool.tile([P, 4, C], bf16, tag="xr", name=f"xr{ci}")
                    nc.scalar.dma_start(xrc, xr_v[:, ci * 4:(ci + 1) * 4, :])
                    xrs_of[ci] = xrc
                ctx = psE[ci % 2].tile([D + 1, 512], f32, tag="cx", name=f"ctx{cs}")
                for g, grp in enumerate(groups):
                    if first:
                        # fg rides one chunk ahead of QK demand; h pairs are
                        # emitted AFTER the group's QK/EXP so the first EXPs
                        # are never queued behind them on the in-order PE.
                        # two fg chunks per group boundary: consecutive fg
                        # allocs in the sp pool then wait each other's fast
                        # DVE bias reads instead of EXPs, and fg7's chains
                        # issue right as its xt slab lands (~19us)
                        while fg_done <= min(2 * g + 1, nch - 1):
                            emit_fg_chunk(fg_done)
                            fg_done += 1
                    ep = emit_qk_exp(ck, g, offload=not first)
                    pv_q.append((ck, g, ctx, ep))
                    flush_pv()
                    if first:
                        while (8 * h_done < 4 * fg_done
                               and 8 * h_done <= grp[-1] + 8):
                            emit_h_oct(h_done)
                            h_done += 1
                    if pending is not None:
                        pck, pct, ppool = pending
                        if g == 2:
                            rcp = emit_denoms(pck, pct, ppool)
                        elif 3 <= g <= len(pck[2]) + 2:
                            emit_out_tile(pck, g - 3, pct, rcp, ppool)
                            if g == len(pck[2]) + 2:
                                pending = None
                if first:
                    xrc = xrpool.tile([P, 4, C], bf16, tag="xr", name="xr0")
                    nc.scalar.dma_start(xrc, xr_v[:, 0:4, :])
                    xrs_of[0] = xrc
            flush_pv(keep=0)
            pck, pct, ppool = pending
            rcp = emit_denoms(pck, pct, ppool)
            for t in range(len(pck[2])):
                emit_out_tile(pck, t, pct, rcp, psE[t % 2], tail=True)

    nc.compile()
    return nc


def get_program(n: int = N_FULL, h_bias_zero: bool = False):
    key = (n, h_bias_zero)
    if key not in _CACHE:
        _CACHE[key] = _build(n, h_bias_zero)
    return _CACHE[key]


def make_weight_maps(Wf, bf, Wg, bg, Wh, bh, Wv, bv, gamma, h_bias_zero=False):
    """Host-side layout prep of the tiny replicated weights."""
    wv_aug = np.concatenate(
        [np.float32(gamma) * np.asarray(Wv, np.float32),
         np.asarray(bv, np.float32)[None, :]], axis=0)
    bfg = np.concatenate(
        [np.asarray(bf, np.float32), np.asarray(bg, np.float32)]).reshape(P, 1)
    wfg = np.concatenate(
        [np.asarray(Wf, np.float32), np.asarray(Wg, np.float32)], axis=1)
    # c index decomposition: c = cc*128 + p  ->  [p, cc, d]
    maps = {
        "wfg": np.ascontiguousarray(
            wfg.astype(BF16).reshape(CC, P, 2 * D).transpose(1, 0, 2)),
        "wh": np.ascontiguousarray(
            np.asarray(Wh, np.float32).astype(BF16).reshape(CC, P, D).transpose(1, 0, 2)),
        "bfg": np.ascontiguousarray(bfg),
        "bhp": np.ascontiguousarray(
            np.asarray(bh, np.float32).astype(BF16).reshape(1, D)),
        "onesp": np.ones((1, P), dtype=BF16),
        "wv": np.ascontiguousarray(wv_aug.astype(BF16)),
    }
    if h_bias_zero:
        del maps["bhp"], maps["onesp"]
    return maps


def make_x_maps(xf_b):
    """Per-core x layouts: residual rows (bf16) + transposed bf16 [p, cc, n]."""
    x = np.ascontiguousarray(xf_b, dtype=np.float32)
    xt = x.T.astype(BF16).reshape(CC, P, x.shape[0]).transpose(1, 0, 2)
    return {"xr": x.astype(BF16), "xt": np.ascontiguousarray(xt)}


def kernel(x, Wf, bf, Wg, bg, Wh, bh, Wv, bv, gamma):
    from concourse.bass_utils import run_bass_kernel_spmd

    x = np.asarray(x, np.float32)
    b, hh, ww, c = x.shape
    n = hh * ww
    assert (b, c) == (B, C)

    hbz = bool(np.all(np.asarray(bh) == 0))
    nc = get_program(n, hbz)
    base = make_weight_maps(Wf, bf, Wg, bg, Wh, bh, Wv, bv, gamma, hbz)
    xf = x.reshape(b, n, c)
    in_maps = [dict(base, **make_x_maps(xf[i])) for i in range(b)]

    res = run_bass_kernel_spmd(nc, in_maps, core_ids=list(range(b)))
    out = np.stack([np.asarray(res.results[i]["out"], np.float32)
                    for i in range(b)], axis=0)
    return np.ascontiguousarray(out.reshape(b, hh, ww, c).astype(np.float32))


# revision 44
# speedup vs baseline: 1.0722x; 1.0293x over previous
"""SAGAN self-attention block on 8 TRN2 NeuronCores (v7, ~174-176us; v4 was 202us).

Reference (per batch element b, N = H*W = 4096, C = 512, D = 64):
    f = x @ Wf + bf ; g = x @ Wg + bg ; h = x @ Wh + bh      # [N, D]
    s = f @ g.T                                              # [N, N]
    attn = softmax(s, axis=-1)
    ctx = attn @ h                                           # [N, D]
    o = (gamma * ctx) @ Wv + bv + x                          # [N, C]

Sharding: data-parallel over batch B=8 -> one batch element per core, no
collectives. Weights replicated.

Device algorithm (per core), bf16 matmuls with f32 PSUM accumulation:
  - s is computed per m-tile (keys on partitions) in 3-m-tile groups; QK
    pairs row-pack via the FG2/GF2 stacked+mirrored f/g tensors (K=64
    streams 2 cols/cycle with tile_position).
  - softmax is unnormalized (no max subtraction); denominators ride as a
    ones-column in haug through the PV accumulation, are PE-transposed to
    per-partition scalars, and one DVE reciprocal per chunk feeds the
    fused (out*rc + x) epilogue.
  - EXP of the 16.7M logits is the ScalarE bottleneck (0.833ns/elem/lane
    = 109us floor), so the 3rd tile of each triple group is offloaded to
    the DVE as a Schraudolph exp: bf16_bits(e^s) ~= s*(2^7/ln2) + B in
    int16, one tensor_scalar per tile.  gamma=0.01 makes the attention
    term only ~0.8%% of the output norm, so the ~3%% max rel err of the
    approximation costs ~3e-5 end-to-end (gate is 2e-2).
  - PV is software-pipelined two groups behind QK/EXP so the DVE exp hop
    is never on the PE critical path; ctx PSUM ping-pongs between banks
    6/7 per chunk; epilogue (denoms at g==2, out-proj at g 3..6) rides
    inside the next chunk.
  - residual x rows and the output stream are bf16 (12.3MB DMA/core);
    DMA is spread over the two HWDGE queues (Sync: first-chunk slices,
    f/g mirrors, xr slabs; Act: weights + bulk xt slabs) plus GPSIMD
    SWDGE for output stores; final stores split 3 ways across queues.
  - chunk 0 JIT-interleaves the f/g/h projections with its QK groups;
    h accumulates in a PSUM arena in the idle odd ctx bank in rounds of
    8 chains + one CAST (the conservative tile-granular WAR between arena
    writers and CAST readers then bites once per 8 tiles, not per chain).
  - the f/g mirrors ride the GPSIMD SWDGE queue, pre-warmed by the bfg
    load (cold start ~10us), so both HWDGE queues carry xt slabs and all
    of xt lands ~7us earlier.
"""

import numpy as np
import ml_dtypes

BF16 = ml_dtypes.bfloat16

B, HH, WW, C = 8, 64, 64, 512
D = C // 8          # 64
N_FULL = HH * WW    # 4096
P = 128
CC = C // P         # 4  (c-chunks of 128)

_CACHE: dict = {}


def _groups(n_tiles):
    """m-tile groups per n-chunk: triples + a final pair (e.g. 10x3 + 1x2)."""
    gs = []
    i = 0
    while n_tiles - i >= 3:
        if n_tiles - i == 4:
            break
        gs.append([i, i + 1, i + 2])
        i += 3
    while i < n_tiles:
        gs.append(list(range(i, min(i + 2, n_tiles))))
        i += 2
    return gs


def _build(n: int, h_bias_zero: bool = False):
    import concourse.mybir as mybir
    from concourse import bacc
    from concourse.tile import TileContext

    f32 = mybir.dt.float32
    bf16 = mybir.dt.bfloat16
    i16 = mybir.dt.int16
    # Schraudolph exp in bf16-bit space: bf16_bits(exp(s)) ~= s*(2^7/ln2) + B.
    # One DVE tensor_scalar (f32 PSUM -> int16 SBUF) computes a ~3% max-rel-err
    # exp; with gamma=0.01 scaling the attention term, the end-to-end error is
    # ~3e-5.  Used for the 3rd tile of each triple group outside chunk 0 to
    # offload ~1/3 of the softmax EXP stream from the ScalarE bottleneck.
    EXP_A = float(128.0 / np.log(2.0))
    EXP_B = 16250.625
    ADD = mybir.AluOpType.add
    MULT = mybir.AluOpType.mult
    EXP = mybir.ActivationFunctionType.Exp

    n_tiles = n // P        # 32
    nch = n // 512          # 8

    nc = bacc.Bacc("TRN2", target_bir_lowering=False, debug=False)

    xr_d = nc.dram_tensor("xr", [n, C], bf16, kind="ExternalInput")
    xt_d = nc.dram_tensor("xt", [P, CC, n], bf16, kind="ExternalInput")
    wfg_d = nc.dram_tensor("wfg", [P, CC, 2 * D], bf16, kind="ExternalInput")
    wh_d = nc.dram_tensor("wh", [P, CC, D], bf16, kind="ExternalInput")
    bfg_d = nc.dram_tensor("bfg", [P, 1], f32, kind="ExternalInput")   # [bf;bg]
    if not h_bias_zero:
        bh_d = nc.dram_tensor("bhp", [1, D], bf16, kind="ExternalInput")
        on_d = nc.dram_tensor("onesp", [1, P], bf16, kind="ExternalInput")
    wv_d = nc.dram_tensor("wv", [D + 1, C], bf16, kind="ExternalInput")
    out_d = nc.dram_tensor("out", [n, C], bf16, kind="ExternalOutput")

    xr_v = xr_d.rearrange("(i p) c -> p i c", p=P)
    o_t = out_d.rearrange("(i p) c -> i p c", p=P)

    groups = _groups(n_tiles)
    need_fg = [grp[-1] // 4 for grp in groups]

    with TileContext(nc) as tc:
        with (
            tc.tile_pool(name="const", bufs=1) as cpool,
            tc.tile_pool(name="big", bufs=1) as bigpool,
            tc.tile_pool(name="ep", bufs=5) as epool,
            tc.tile_pool(name="eps", bufs=5) as epspool,
            tc.tile_pool(name="ct", bufs=2) as ctpool,
            tc.tile_pool(name="os", bufs=4) as opool,
            tc.tile_pool(name="xr", bufs=8) as xrpool,
            tc.tile_pool(name="sm", bufs=4) as smpool,
            tc.tile_pool(name="psSP", bufs=2, space="PSUM") as psSP,
            tc.tile_pool(name="psS3", bufs=2, space="PSUM") as psS3,
            tc.tile_pool(name="psE0", bufs=1, space="PSUM") as psE0,
            tc.tile_pool(name="psE1", bufs=1, space="PSUM") as psE1,
        ):
            psE = [psE0, psE1]

            # ---- replicated constants -> SBUF (small ones on qAct)
            # bfg rides the GPSIMD SWDGE queue first to warm it up: the f/g
            # mirrors use it mid-prologue and its cold start is ~10us.
            bfg_sb = cpool.tile([P, 1], f32)
            nc.gpsimd.dma_start(bfg_sb, bfg_d[:, :])
            wfg_sb = cpool.tile([P, CC, 2 * D], bf16)
            nc.scalar.dma_start(wfg_sb, wfg_d[:, :, :])

            # ---- persistent SBUF tensors
            xt = bigpool.tile([P, CC, n], bf16)          # x.T (c on partitions)
            FG2 = bigpool.tile([P, n], bf16)             # rows 0:64 f.T, 64:128 g.T
            GF2 = bigpool.tile([P, n], bf16)             # rows 0:64 g.T, 64:128 f.T
            haug = bigpool.tile([P, n_tiles, D + 1], bf16)
            nc.gpsimd.memset(haug[:, :, D:D + 1], 1.0)

            # xt DMAs: chunk 0+1 split across both HW queues (per-cc slices
            # for fast first-chunk), later chunks as one batched slab each so
            # the Sync sequencer only spends ~0.6us per chunk.
            for jc in range(2):
                for cc in range(CC):
                    sl = slice(jc * 512, (jc + 1) * 512)
                    eng = nc.sync if (cc % 2 == 0) else nc.scalar
                    eng.dma_start(xt[:, cc, sl], xt_d[:, cc, sl])

            wh_sb = cpool.tile([P, CC, D], bf16)
            nc.scalar.dma_start(wh_sb, wh_d[:, :, :])
            if not h_bias_zero:
                bh_sb = cpool.tile([1, D], bf16)
                nc.scalar.dma_start(bh_sb, bh_d[:, :])
                ones_sb = cpool.tile([1, P], bf16)
                nc.scalar.dma_start(ones_sb, on_d[:, :])
            wv_sb = cpool.tile([D + 1, C], bf16)
            nc.scalar.dma_start(wv_sb, wv_d[:, :])

            # bulk xt slabs split across BOTH HWDGE queues (mirrors ride the
            # pre-warmed SWDGE queue, so neither HW queue must stay empty);
            # all of xt lands ~7us earlier, unblocking the tail fg chunks.
            # qSyIo gets only slab-2 so the latency-critical mirror-0/1
            # transfers are never behind bulk xt in its FIFO (the SWDGE
            # queue turned out to be ~4us push-to-data + ~1.5us/transfer,
            # which made QK-g0 wait until ~24.7us)
            for jc in range(2, nch):
                sl = slice(jc * 512, (jc + 1) * 512)
                eng = nc.sync if jc == 2 else nc.scalar
                eng.dma_start(xt[:, :, sl], xt_d[:, :, sl])

            def emit_fg_chunk(jc):
                """f/g projection for 512-chunk jc -> FG2 + GF2 mirror."""
                sl = slice(jc * 512, (jc + 1) * 512)
                # fg chunks 2+ rotate through psS3, whose waits are all
                # DVE-released (schraud/bias), never ACT -- this breaks the
                # chunk-0 PE->ACT->PE stutter ring in the sp pool
                pool, tag = (psSP, "sp") if jc < 2 else (psS3, "s3")
                fgp = pool.tile([P, 512], f32, tag=tag, name=f"fg{jc}")
                for cc in range(CC):
                    nc.tensor.matmul(
                        fgp, lhsT=wfg_sb[:, cc, :], rhs=xt[:, cc, sl],
                        start=(cc == 0), stop=(cc == CC - 1),
                    )
                nc.vector.tensor_scalar(FG2[:, sl], fgp, bfg_sb, None, ADD)
                # mirror halves swapped (latency-critical: QK g0 needs them)
                nc.sync.dma_start(GF2[D:P, sl], FG2[0:D, sl])
                nc.sync.dma_start(GF2[0:D, sl], FG2[D:P, sl])

            # h arena: 8 rotating [128, 64] slots in PSUM bank 7 (psE1's bank
            # is otherwise unused until ctx_1 at chunk 1).
            hparena = psE1.tile([P, 512], f32, tag="cx", name="hparena")

            def emit_h_oct(r):
                """h projection for m-tiles 8r..8r+7: 8 accumulation chains
                into the arena, then ONE CAST into haug.  All writes precede
                the single read, so the conservative tile-granular WAR between
                arena writers and haug-CAST readers bites once per 8 tiles
                instead of once per chain."""
                for t in range(8):
                    i = 8 * r + t
                    hp = hparena[:, t * D:(t + 1) * D]
                    for cc in range(CC):
                        nc.tensor.matmul(
                            hp, lhsT=xt[:, cc, i * P:(i + 1) * P],
                            rhs=wh_sb[:, cc, :],
                            start=(cc == 0), stop=(h_bias_zero and cc == CC - 1),
                        )
                    if not h_bias_zero:
                        nc.tensor.matmul(
                            hp, lhsT=ones_sb, rhs=bh_sb, start=False, stop=True)
                # CAST on the ScalarE: it is idle until its first EXP, while
                # the DVE carries the bias->mirror chain that gates QK
                nc.scalar.copy(
                    haug[:, 8 * r:8 * r + 8, 0:D],
                    hparena.rearrange("p (t d) -> p t d", d=D))

            def emit_qk_exp(ck, g, offload=False):
                """QK + EXP for m-tile group g of column-chunk ck.
                Tiles 0-1 go to the 2-bank sp pool (released by the ScalarE
                EXP alone); tile 2 goes to its own 1-bank pool (released by
                the DVE Schraudolph alone), so a lag on either exp engine
                no longer stalls BOTH QK allocations two groups later.
                Returns (ep, ep2, ep2_is_i16)."""
                cs, cw, _ = ck
                sl = slice(cs, cs + cw)
                grp = groups[g]

                def qk(dst, q, i):
                    # row-pack QK by m-tile parity: even tiles use rows 0:64
                    # (g in GF2, f in FG2), odd tiles rows 64:128.
                    hb = (i % 2) * D
                    lhs = GF2 if hb == 0 else FG2
                    rhs = FG2 if hb == 0 else GF2
                    nc.tensor.matmul(
                        dst[:, q * 512:q * 512 + cw],
                        lhsT=lhs[hb:hb + D, i * P:(i + 1) * P],
                        rhs=rhs[hb:hb + D, sl],
                        start=True, stop=True, tile_position=(hb, 0),
                    )

                sp = psSP.tile([P, 1024], f32, tag="sp", name=f"sp{cs}_{g}")
                qk(sp, 0, grp[0])
                if len(grp) > 1:
                    qk(sp, 1, grp[1])
                sp3 = None
                if len(grp) == 3:
                    sp3 = psS3.tile([P, 512], f32, tag="s3", name=f"s3{cs}_{g}")
                    qk(sp3, 0, grp[2])
                ep = epool.tile([P, 1024], bf16, tag="ep")
                spv = sp.rearrange("p (q v) -> p q v", v=512)
                epv = ep.rearrange("p (q v) -> p q v", v=512)
                nw = min(2, len(grp))
                nc.scalar.activation(epv[:, 0:nw, 0:cw], spv[:, 0:nw, 0:cw], EXP)
                if sp3 is None:
                    return ep, None, False
                if offload:
                    epS = epspool.tile([P, 512], i16, tag="eps")
                    nc.vector.tensor_scalar(
                        epS[:, 0:cw], sp3[:, 0:cw], EXP_A, EXP_B, MULT, ADD)
                    return ep, epS, True
                ep3 = epspool.tile([P, 512], bf16, tag="eps")
                nc.scalar.activation(ep3[:, 0:cw], sp3[:, 0:cw], EXP)
                return ep, ep3, False

            def emit_pv(ck, g, ctx, eps):
                ep, ep2, is_i16 = eps
                cs, cw, _ = ck
                for q, i in enumerate(groups[g]):
                    if q == 2:
                        rhs = (ep2.bitcast(bf16) if is_i16 else ep2)[:, 0:cw]
                    else:
                        rhs = ep[:, q * 512:q * 512 + cw]
                    nc.tensor.matmul(
                        ctx[:, 0:cw], lhsT=haug[:, i, :], rhs=rhs,
                        start=(g == 0 and q == 0), stop=(i == n_tiles - 1),
                    )

            def emit_ct_copy(ck, ctx):
                """ctx -> bf16 SBUF copy (DVE)."""
                cs, cw, _ = ck
                ct = ctpool.tile([D + 1, 512], bf16, tag="ct", name=f"ct{cs}")
                nc.vector.tensor_copy(out=ct[:, 0:cw], in_=ctx[:, 0:cw])
                return ct

            def emit_denoms(ck, ct, pool):
                """PE transposes of the denominator row into one PSUM tile in
                the pending chunk's ping-pong bank, then one DVE reciprocal."""
                _, cw, tiles = ck
                # bf16 PSUM writes need 4-byte alignment: space columns 2 apart
                dt4 = pool.tile([P, 8], bf16, tag="cx", name=f"dt{tiles[0]}")
                for t in range(len(tiles)):
                    tsl = slice(t * P, (t + 1) * P)
                    nc.tensor.transpose(
                        dt4[:, 2 * t:2 * t + 1], ct[D:D + 1, tsl],
                        haug[D:D + 1, 0, D:D + 1])
                rc4 = smpool.tile([P, 8], f32, tag="rc")
                nc.vector.reciprocal(rc4, dt4)
                return rc4

            def emit_out_tile(ck, t, ct, rc, pool, tail=False):
                """out-proj + scale + residual + store for one 128-row tile."""
                it = ck[2][t]
                tsl = slice(t * P, (t + 1) * P)
                op = pool.tile([P, C], f32, tag="cx", name=f"op{it}")
                nc.tensor.matmul(op, lhsT=ct[:, tsl], rhs=wv_sb, start=True, stop=True)
                osb = opool.tile([P, C], bf16, tag="os")
                nc.vector.scalar_tensor_tensor(
                    out=osb, in0=op, scalar=rc[:, 2 * t:2 * t + 1],
                    in1=xrs_of[it // 4][:, it % 4, :], op0=MULT, op1=ADD)
                if tail:
                    # final stores: split across SWDGE + Sync + Act queues so
                    # the drain after the last compute is short
                    nc.gpsimd.dma_start(o_t[it][0:48, :], osb[0:48, :])
                    nc.sync.dma_start(o_t[it][48:96, :], osb[48:96, :])
                    nc.scalar.dma_start(o_t[it][96:P, :], osb[96:P, :])
                else:
                    nc.gpsimd.dma_start(o_t[it], osb)

            # ---- emission schedule -------------------------------------
            chunks = [(j * 512, 512, [4 * j + t for t in range(4)])
                      for j in range(nch)]

            fg_done = 0
            h_done = 0
            xrs_of = {}
            pending = None   # epilogue state: (ck, ct, pool)
            pv_q = []        # deferred PVs: (ck, g, ctx, eps), depth 2
            rcp = None

            def flush_pv(keep=2):
                # PV runs two groups behind its QK/EXP so the DVE-offloaded
                # exp tile is never on the PV critical path
                nonlocal pending
                while len(pv_q) > keep:
                    pck0, g0, ctx0, ep0 = pv_q.pop(0)
                    emit_pv(pck0, g0, ctx0, ep0)
                    if g0 == len(groups) - 1:
                        # chunk-final PV: ctx done -> bf16 copy, open epilogue
                        pending = (pck0, emit_ct_copy(pck0, ctx0),
                                   psE[(pck0[0] // 512) % 2])

            for ci, ck in enumerate(chunks):
                cs, cw, tiles = ck
                first = (ci == 0)
                ctx = psE[ci % 2].tile([D + 1, 512], f32, tag="cx", name=f"ctx{cs}")
                for g, grp in enumerate(groups):
                    if first:
                        # fg rides one chunk ahead of QK demand; h pairs are
                        # emitted AFTER the group's QK/EXP so the first EXPs
                        # are never queued behind them on the in-order PE.
                        # two fg chunks per group boundary: consecutive fg
                        # allocs in the sp pool then wait each other's fast
                        # DVE bias reads instead of EXPs, and fg7's chains
                        # issue right as its xt slab lands (~19us)
                        while fg_done <= min(2 * g + 1, nch - 1):
                            emit_fg_chunk(fg_done)
                            fg_done += 1
                    ep = emit_qk_exp(ck, g, offload=not first)
                    pv_q.append((ck, g, ctx, ep))
                    flush_pv()
                    if first:
                        while (8 * h_done < 4 * fg_done
                               and 8 * h_done <= grp[-1] + 8):
                            emit_h_oct(h_done)
                            h_done += 1
                    if pending is not None:
                        pck, pct, ppool = pending
                        if g == 2:
                            rcp = emit_denoms(pck, pct, ppool)
                        elif 3 <= g <= len(pck[2]) + 2:
                            emit_out_tile(pck, g - 3, pct, rcp, ppool)
                            if g == len(pck[2]) + 2:
                                pending = None
                # residual rows (consumed by this chunk's epilogue during
                # the next chunk), emitted at chunk END so the scheduler's
                # frozen Sync order cannot interleave them between the
                # latency-critical mirror halves during chunk 0
                xrc = xrpool.tile([P, 4, C], bf16, tag="xr", name=f"xr{ci}")
                nc.sync.dma_start(xrc, xr_v[:, ci * 4:(ci + 1) * 4, :])
                xrs_of[ci] = xrc
            flush_pv(keep=0)
            pck, pct, ppool = pending
            rcp = emit_denoms(pck, pct, ppool)
            for t in range(len(pck[2])):
                emit_out_tile(pck, t, pct, rcp, psE[t % 2], tail=True)

    nc.compile()
    return nc


def get_program(n: int = N_FULL, h_bias_zero: bool = False):
    key = (n, h_bias_zero)
    if key not in _CACHE:
        _CACHE[key] = _build(n, h_bias_zero)
    return _CACHE[key]


def make_weight_maps(Wf, bf, Wg, bg, Wh, bh, Wv, bv, gamma, h_bias_zero=False):
    """Host-side layout prep of the tiny replicated weights."""
    wv_aug = np.concatenate(
        [np.float32(gamma) * np.asarray(Wv, np.float32),
         np.asarray(bv, np.float32)[None, :]], axis=0)
    bfg = np.concatenate(
        [np.asarray(bf, np.float32), np.asarray(bg, np.float32)]).reshape(P, 1)
    wfg = np.concatenate(
        [np.asarray(Wf, np.float32), np.asarray(Wg, np.float32)], axis=1)
    # c index decomposition: c = cc*128 + p  ->  [p, cc, d]
    maps = {
        "wfg": np.ascontiguousarray(
            wfg.astype(BF16).reshape(CC, P, 2 * D).transpose(1, 0, 2)),
        "wh": np.ascontiguousarray(
            np.asarray(Wh, np.float32).astype(BF16).reshape(CC, P, D).transpose(1, 0, 2)),
        "bfg": np.ascontiguousarray(bfg),
        "bhp": np.ascontiguousarray(
            np.asarray(bh, np.float32).astype(BF16).reshape(1, D)),
        "onesp": np.ones((1, P), dtype=BF16),
        "wv": np.ascontiguousarray(wv_aug.astype(BF16)),
    }
    if h_bias_zero:
        del maps["bhp"], maps["onesp"]
    return maps


def make_x_maps(xf_b):
    """Per-core x layouts: residual rows (bf16) + transposed bf16 [p, cc, n]."""
    x = np.ascontiguousarray(xf_b, dtype=np.float32)
    xt = x.T.astype(BF16).reshape(CC, P, x.shape[0]).transpose(1, 0, 2)
    return {"xr": x.astype(BF16), "xt": np.ascontiguousarray(xt)}


def kernel(x, Wf, bf, Wg, bg, Wh, bh, Wv, bv, gamma):
    from concourse.bass_utils import run_bass_kernel_spmd

    x = np.asarray(x, np.float32)
    b, hh, ww, c = x.shape
    n = hh * ww
    assert (b, c) == (B, C)

    hbz = bool(np.all(np.asarray(bh) == 0))
    nc = get_program(n, hbz)
    base = make_weight_maps(Wf, bf, Wg, bg, Wh, bh, Wv, bv, gamma, hbz)
    xf = x.reshape(b, n, c)
    in_maps = [dict(base, **make_x_maps(xf[i])) for i in range(b)]

    res = run_bass_kernel_spmd(nc, in_maps, core_ids=list(range(b)))
    out = np.stack([np.asarray(res.results[i]["out"], np.float32)
                    for i in range(b)], axis=0)
    return np.ascontiguousarray(out.reshape(b, hh, ww, c).astype(np.float32))


# revision 47
# speedup vs baseline: 1.2381x; 1.1547x over previous
"""SAGAN self-attention block on 8 TRN2 NeuronCores (v7, ~174-176us; v4 was 202us).

Reference (per batch element b, N = H*W = 4096, C = 512, D = 64):
    f = x @ Wf + bf ; g = x @ Wg + bg ; h = x @ Wh + bh      # [N, D]
    s = f @ g.T                                              # [N, N]
    attn = softmax(s, axis=-1)
    ctx = attn @ h                                           # [N, D]
    o = (gamma * ctx) @ Wv + bv + x                          # [N, C]

Sharding: data-parallel over batch B=8 -> one batch element per core, no
collectives. Weights replicated.

Device algorithm (per core), bf16 matmuls with f32 PSUM accumulation:
  - s is computed per m-tile (keys on partitions) in 3-m-tile groups; QK
    pairs row-pack via the FG2/GF2 stacked+mirrored f/g tensors (K=64
    streams 2 cols/cycle with tile_position).
  - softmax is unnormalized (no max subtraction); denominators ride as a
    ones-column in haug through the PV accumulation, are PE-transposed to
    per-partition scalars, and one DVE reciprocal per chunk feeds the
    fused (out*rc + x) epilogue.
  - EXP of the 16.7M logits is the ScalarE bottleneck (0.833ns/elem/lane
    = 109us floor), so the 3rd tile of each triple group is offloaded to
    the DVE as a Schraudolph exp: bf16_bits(e^s) ~= s*(2^7/ln2) + B in
    int16, one tensor_scalar per tile.  gamma=0.01 makes the attention
    term only ~0.8%% of the output norm, so the ~3%% max rel err of the
    approximation costs ~3e-5 end-to-end (gate is 2e-2).
  - PV is software-pipelined two groups behind QK/EXP so the DVE exp hop
    is never on the PE critical path; ctx PSUM ping-pongs between banks
    6/7 per chunk; epilogue (denoms at g==2, out-proj at g 3..6) rides
    inside the next chunk.
  - residual x rows and the output stream are bf16 (12.3MB DMA/core);
    DMA is spread over the two HWDGE queues (Sync: first-chunk slices,
    f/g mirrors, xr slabs; Act: weights + bulk xt slabs) plus GPSIMD
    SWDGE for output stores; final stores split 3 ways across queues.
  - chunk 0 JIT-interleaves the f/g/h projections with its QK groups;
    h accumulates in a PSUM arena in the idle odd ctx bank in rounds of
    8 chains + one CAST (the conservative tile-granular WAR between arena
    writers and CAST readers then bites once per 8 tiles, not per chain).
  - the f/g mirrors ride the GPSIMD SWDGE queue, pre-warmed by the bfg
    load (cold start ~10us), so both HWDGE queues carry xt slabs and all
    of xt lands ~7us earlier.
"""

import numpy as np
import ml_dtypes

BF16 = ml_dtypes.bfloat16

B, HH, WW, C = 8, 64, 64, 512
D = C // 8          # 64
N_FULL = HH * WW    # 4096
P = 128
CC = C // P         # 4  (c-chunks of 128)

_CACHE: dict = {}


def _groups(n_tiles):
    """m-tile groups per n-chunk: triples + a final pair (e.g. 10x3 + 1x2)."""
    gs = []
    i = 0
    while n_tiles - i >= 3:
        if n_tiles - i == 4:
            break
        gs.append([i, i + 1, i + 2])
        i += 3
    while i < n_tiles:
        gs.append(list(range(i, min(i + 2, n_tiles))))
        i += 2
    return gs


def _build(n: int, h_bias_zero: bool = False):
    import concourse.mybir as mybir
    from concourse import bacc
    from concourse.tile import TileContext

    f32 = mybir.dt.float32
    bf16 = mybir.dt.bfloat16
    i16 = mybir.dt.int16
    # Schraudolph exp in bf16-bit space: bf16_bits(exp(s)) ~= s*(2^7/ln2) + B.
    # One DVE tensor_scalar (f32 PSUM -> int16 SBUF) computes a ~3% max-rel-err
    # exp; with gamma=0.01 scaling the attention term, the end-to-end error is
    # ~3e-5.  Used for the 3rd tile of each triple group outside chunk 0 to
    # offload ~1/3 of the softmax EXP stream from the ScalarE bottleneck.
    EXP_A = float(128.0 / np.log(2.0))
    EXP_B = 16250.625
    ADD = mybir.AluOpType.add
    MULT = mybir.AluOpType.mult
    EXP = mybir.ActivationFunctionType.Exp

    n_tiles = n // P        # 32
    nch = n // 512          # 8

    nc = bacc.Bacc("TRN2", target_bir_lowering=False, debug=False)

    xr_d = nc.dram_tensor("xr", [n, C], bf16, kind="ExternalInput")
    xt_d = nc.dram_tensor("xt", [P, CC, n], bf16, kind="ExternalInput")
    wfg_d = nc.dram_tensor("wfg", [P, CC, 2 * D], bf16, kind="ExternalInput")
    wh_d = nc.dram_tensor("wh", [P, CC, D], bf16, kind="ExternalInput")
    bfg_d = nc.dram_tensor("bfg", [P, 1], f32, kind="ExternalInput")   # [bf;bg]
    if not h_bias_zero:
        bh_d = nc.dram_tensor("bhp", [1, D], bf16, kind="ExternalInput")
        on_d = nc.dram_tensor("onesp", [1, P], bf16, kind="ExternalInput")
    wv_d = nc.dram_tensor("wv", [D + 1, C], bf16, kind="ExternalInput")
    out_d = nc.dram_tensor("out", [n, C], bf16, kind="ExternalOutput")

    xr_v = xr_d.rearrange("(i p) c -> p i c", p=P)
    o_t = out_d.rearrange("(i p) c -> i p c", p=P)

    groups = _groups(n_tiles)
    need_fg = [grp[-1] // 4 for grp in groups]

    with TileContext(nc) as tc:
        with (
            tc.tile_pool(name="const", bufs=1) as cpool,
            tc.tile_pool(name="big", bufs=1) as bigpool,
            tc.tile_pool(name="ep", bufs=5) as epool,
            tc.tile_pool(name="eps", bufs=5) as epspool,
            tc.tile_pool(name="ct", bufs=2) as ctpool,
            tc.tile_pool(name="os", bufs=4) as opool,
            tc.tile_pool(name="xr", bufs=8) as xrpool,
            tc.tile_pool(name="sm", bufs=4) as smpool,
            tc.tile_pool(name="psSP", bufs=2, space="PSUM") as psSP,
            tc.tile_pool(name="psS3", bufs=2, space="PSUM") as psS3,
            tc.tile_pool(name="psE0", bufs=1, space="PSUM") as psE0,
            tc.tile_pool(name="psE1", bufs=1, space="PSUM") as psE1,
        ):
            psE = [psE0, psE1]

            # ---- replicated constants -> SBUF (small ones on qAct)
            # bfg rides the GPSIMD SWDGE queue first to warm it up: the f/g
            # mirrors use it mid-prologue and its cold start is ~10us.
            bfg_sb = cpool.tile([P, 1], f32)
            nc.gpsimd.dma_start(bfg_sb, bfg_d[:, :])
            wfg_sb = cpool.tile([P, CC, 2 * D], bf16)
            nc.scalar.dma_start(wfg_sb, wfg_d[:, :, :])

            # ---- persistent SBUF tensors
            xt = bigpool.tile([P, CC, n], bf16)          # x.T (c on partitions)
            FG2 = bigpool.tile([P, n], bf16)             # rows 0:64 f.T, 64:128 g.T
            GF2 = bigpool.tile([P, n], bf16)             # rows 0:64 g.T, 64:128 f.T
            haug = bigpool.tile([P, n_tiles, D + 1], bf16)
            nc.gpsimd.memset(haug[:, :, D:D + 1], 1.0)

            # PE p-state warm-up: ~4us of dummy matmuls on a memset tile
            # while xt chunk 0 is still in flight, so fg0/bias/mirror/QK-g0
            # run at full clock (cold PE is 2-4x slower for its first ~3us)
            warm_in = cpool.tile([P, 16], bf16)
            nc.gpsimd.memset(warm_in, 0.0)
            warm_out = psS3.tile([P, 16], f32, tag="s3", name="warm")
            for _ in range(40):
                nc.tensor.matmul(warm_out[0:16, :], lhsT=warm_in, rhs=warm_in,
                                 start=True, stop=True)

            # xt DMAs: chunk 0+1 split across both HW queues (per-cc slices
            # for fast first-chunk), later chunks as one batched slab each so
            # the Sync sequencer only spends ~0.6us per chunk.
            for jc in range(2):
                for cc in range(CC):
                    sl = slice(jc * 512, (jc + 1) * 512)
                    eng = nc.sync if (cc % 2 == 0) else nc.scalar
                    eng.dma_start(xt[:, cc, sl], xt_d[:, cc, sl])

            wh_sb = cpool.tile([P, CC, D], bf16)
            nc.scalar.dma_start(wh_sb, wh_d[:, :, :])
            if not h_bias_zero:
                bh_sb = cpool.tile([1, D], bf16)
                nc.scalar.dma_start(bh_sb, bh_d[:, :])
                ones_sb = cpool.tile([1, P], bf16)
                nc.scalar.dma_start(ones_sb, on_d[:, :])
            wv_sb = cpool.tile([D + 1, C], bf16)
            nc.scalar.dma_start(wv_sb, wv_d[:, :])

            # bulk xt slabs split across BOTH HWDGE queues (mirrors ride the
            # pre-warmed SWDGE queue, so neither HW queue must stay empty);
            # all of xt lands ~7us earlier, unblocking the tail fg chunks.
            # qSyIo gets only slab-2 so the latency-critical mirror-0/1
            # transfers are never behind bulk xt in its FIFO (the SWDGE
            # queue turned out to be ~4us push-to-data + ~1.5us/transfer,
            # which made QK-g0 wait until ~24.7us)
            for jc in range(2, nch):
                sl = slice(jc * 512, (jc + 1) * 512)
                eng = nc.sync if jc == 2 else nc.scalar
                eng.dma_start(xt[:, :, sl], xt_d[:, :, sl])

            def emit_fg_chunk(jc):
                """f/g projection for 512-chunk jc -> FG2 + GF2 mirror."""
                sl = slice(jc * 512, (jc + 1) * 512)
                # fg chunks 2+ rotate through psS3, whose waits are all
                # DVE-released (schraud/bias), never ACT -- this breaks the
                # chunk-0 PE->ACT->PE stutter ring in the sp pool
                pool, tag = (psSP, "sp") if jc < 2 else (psS3, "s3")
                fgp = pool.tile([P, 512], f32, tag=tag, name=f"fg{jc}")
                for cc in range(CC):
                    nc.tensor.matmul(
                        fgp, lhsT=wfg_sb[:, cc, :], rhs=xt[:, cc, sl],
                        start=(cc == 0), stop=(cc == CC - 1),
                    )
                nc.vector.tensor_scalar(FG2[:, sl], fgp, bfg_sb, None, ADD)
                # mirror halves swapped (latency-critical: QK g0 needs them)
                nc.sync.dma_start(GF2[D:P, sl], FG2[0:D, sl])
                nc.sync.dma_start(GF2[0:D, sl], FG2[D:P, sl])

            # h arena: 8 rotating [128, 64] slots in PSUM bank 7 (psE1's bank
            # is otherwise unused until ctx_1 at chunk 1).
            hparena = psE1.tile([P, 512], f32, tag="cx", name="hparena")

            def emit_h_oct(r):
                """h projection for m-tiles 8r..8r+7: 8 accumulation chains
                into the arena, then ONE CAST into haug.  All writes precede
                the single read, so the conservative tile-granular WAR between
                arena writers and haug-CAST readers bites once per 8 tiles
                instead of once per chain."""
                for t in range(8):
                    i = 8 * r + t
                    hp = hparena[:, t * D:(t + 1) * D]
                    for cc in range(CC):
                        nc.tensor.matmul(
                            hp, lhsT=xt[:, cc, i * P:(i + 1) * P],
                            rhs=wh_sb[:, cc, :],
                            start=(cc == 0), stop=(h_bias_zero and cc == CC - 1),
                        )
                    if not h_bias_zero:
                        nc.tensor.matmul(
                            hp, lhsT=ones_sb, rhs=bh_sb, start=False, stop=True)
                # CAST on the ScalarE: it is idle until its first EXP, while
                # the DVE carries the bias->mirror chain that gates QK
                nc.scalar.copy(
                    haug[:, 8 * r:8 * r + 8, 0:D],
                    hparena.rearrange("p (t d) -> p t d", d=D))

            def emit_qk_exp(ck, g, offload=False):
                """QK + EXP for m-tile group g of column-chunk ck.
                Tiles 0-1 go to the 2-bank sp pool (released by the ScalarE
                EXP alone); tile 2 goes to its own 1-bank pool (released by
                the DVE Schraudolph alone), so a lag on either exp engine
                no longer stalls BOTH QK allocations two groups later.
                Returns (ep, ep2, ep2_is_i16)."""
                cs, cw, _ = ck
                sl = slice(cs, cs + cw)
                grp = groups[g]

                def qk(dst, q, i):
                    # row-pack QK by m-tile parity: even tiles use rows 0:64
                    # (g in GF2, f in FG2), odd tiles rows 64:128.
                    hb = (i % 2) * D
                    lhs = GF2 if hb == 0 else FG2
                    rhs = FG2 if hb == 0 else GF2
                    nc.tensor.matmul(
                        dst[:, q * 512:q * 512 + cw],
                        lhsT=lhs[hb:hb + D, i * P:(i + 1) * P],
                        rhs=rhs[hb:hb + D, sl],
                        start=True, stop=True, tile_position=(hb, 0),
                    )

                sp = psSP.tile([P, 1024], f32, tag="sp", name=f"sp{cs}_{g}")
                qk(sp, 0, grp[0])
                if len(grp) > 1:
                    qk(sp, 1, grp[1])
                sp3 = None
                if len(grp) == 3:
                    sp3 = psS3.tile([P, 512], f32, tag="s3", name=f"s3{cs}_{g}")
                    qk(sp3, 0, grp[2])
                ep = epool.tile([P, 1024], bf16, tag="ep")
                spv = sp.rearrange("p (q v) -> p q v", v=512)
                epv = ep.rearrange("p (q v) -> p q v", v=512)
                nw = min(2, len(grp))
                nc.scalar.activation(epv[:, 0:nw, 0:cw], spv[:, 0:nw, 0:cw], EXP)
                if sp3 is None:
                    return ep, None, False
                if offload:
                    epS = epspool.tile([P, 512], i16, tag="eps")
                    nc.vector.tensor_scalar(
                        epS[:, 0:cw], sp3[:, 0:cw], EXP_A, EXP_B, MULT, ADD)
                    return ep, epS, True
                ep3 = epspool.tile([P, 512], bf16, tag="eps")
                nc.scalar.activation(ep3[:, 0:cw], sp3[:, 0:cw], EXP)
                return ep, ep3, False

            def emit_pv(ck, g, ctx, eps):
                ep, ep2, is_i16 = eps
                cs, cw, _ = ck
                for q, i in enumerate(groups[g]):
                    if q == 2:
                        rhs = (ep2.bitcast(bf16) if is_i16 else ep2)[:, 0:cw]
                    else:
                        rhs = ep[:, q * 512:q * 512 + cw]
                    nc.tensor.matmul(
                        ctx[:, 0:cw], lhsT=haug[:, i, :], rhs=rhs,
                        start=(g == 0 and q == 0), stop=(i == n_tiles - 1),
                    )

            def emit_ct_copy(ck, ctx):
                """ctx -> bf16 SBUF copy (DVE)."""
                cs, cw, _ = ck
                ct = ctpool.tile([D + 1, 512], bf16, tag="ct", name=f"ct{cs}")
                nc.vector.tensor_copy(out=ct[:, 0:cw], in_=ctx[:, 0:cw])
                return ct

            def emit_denoms(ck, ct, pool):
                """PE transposes of the denominator row into one PSUM tile in
                the pending chunk's ping-pong bank, then one DVE reciprocal."""
                _, cw, tiles = ck
                # bf16 PSUM writes need 4-byte alignment: space columns 2 apart
                dt4 = pool.tile([P, 8], bf16, tag="cx", name=f"dt{tiles[0]}")
                for t in range(len(tiles)):
                    tsl = slice(t * P, (t + 1) * P)
                    nc.tensor.transpose(
                        dt4[:, 2 * t:2 * t + 1], ct[D:D + 1, tsl],
                        haug[D:D + 1, 0, D:D + 1])
                rc4 = smpool.tile([P, 8], f32, tag="rc")
                nc.vector.reciprocal(rc4, dt4)
                return rc4

            def emit_out_tile(ck, t, ct, rc, pool, tail=False):
                """out-proj + scale + residual + store for one 128-row tile."""
                it = ck[2][t]
                tsl = slice(t * P, (t + 1) * P)
                op = pool.tile([P, C], f32, tag="cx", name=f"op{it}")
                nc.tensor.matmul(op, lhsT=ct[:, tsl], rhs=wv_sb, start=True, stop=True)
                osb = opool.tile([P, C], bf16, tag="os")
                nc.vector.scalar_tensor_tensor(
                    out=osb, in0=op, scalar=rc[:, 2 * t:2 * t + 1],
                    in1=xrs_of[it // 4][:, it % 4, :], op0=MULT, op1=ADD)
                if tail:
                    # final stores: both HW queues are idle at the tail and
                    # far faster than SWDGE (~1.5us/transfer)
                    nc.sync.dma_start(o_t[it][0:D, :], osb[0:D, :])
                    nc.scalar.dma_start(o_t[it][D:P, :], osb[D:P, :])
                else:
                    nc.gpsimd.dma_start(o_t[it], osb)

            # ---- emission schedule -------------------------------------
            chunks = [(j * 512, 512, [4 * j + t for t in range(4)])
                      for j in range(nch)]

            fg_done = 0
            h_done = 0
            xrs_of = {}
            pending = None   # epilogue state: (ck, ct, pool)
            pv_q = []        # deferred PVs: (ck, g, ctx, eps), depth 2
            rcp = None

            def flush_pv(keep=2):
                # PV runs two groups behind its QK/EXP so the DVE-offloaded
                # exp tile is never on the PV critical path
                nonlocal pending
                while len(pv_q) > keep:
                    pck0, g0, ctx0, ep0 = pv_q.pop(0)
                    emit_pv(pck0, g0, ctx0, ep0)
                    if g0 == len(groups) - 1:
                        # chunk-final PV: ctx done -> bf16 copy, open epilogue
                        pending = (pck0, emit_ct_copy(pck0, ctx0),
                                   psE[(pck0[0] // 512) % 2])

            for ci, ck in enumerate(chunks):
                cs, cw, tiles = ck
                first = (ci == 0)
                if not first:
                    # residual rows (consumed by this chunk's epilogue during
                    # the next chunk); chunk 0's slab is deferred so it does
                    # not delay the latency-critical f/g mirrors on qSyIo.
                    xrc = xrpool.tile([P, 4, C], bf16, tag="xr", name=f"xr{ci}")
                    nc.sync.dma_start(xrc, xr_v[:, ci * 4:(ci + 1) * 4, :])
                    xrs_of[ci] = xrc
                ctx = psE[ci % 2].tile([D + 1, 512], f32, tag="cx", name=f"ctx{cs}")
                for g, grp in enumerate(groups):
                    if first:
                        # fg rides one chunk ahead of QK demand; h pairs are
                        # emitted AFTER the group's QK/EXP so the first EXPs
                        # are never queued behind them on the in-order PE.
                        # two fg chunks per group boundary: consecutive fg
                        # allocs in the sp pool then wait each other's fast
                        # DVE bias reads instead of EXPs, and fg7's chains
                        # issue right as its xt slab lands (~19us)
                        while fg_done <= min(2 * g + 1, nch - 1):
                            emit_fg_chunk(fg_done)
                            fg_done += 1
                    ep = emit_qk_exp(ck, g, offload=not first)
                    pv_q.append((ck, g, ctx, ep))
                    flush_pv()
                    if first:
                        while (8 * h_done < 4 * fg_done
                               and 8 * h_done <= grp[-1] + 8):
                            emit_h_oct(h_done)
                            h_done += 1
                    if pending is not None:
                        pck, pct, ppool = pending
                        if g == 2:
                            rcp = emit_denoms(pck, pct, ppool)
                        elif 3 <= g <= len(pck[2]) + 2:
                            emit_out_tile(pck, g - 3, pct, rcp, ppool)
                            if g == len(pck[2]) + 2:
                                pending = None
                if first:
                    xrc = xrpool.tile([P, 4, C], bf16, tag="xr", name="xr0")
                    nc.sync.dma_start(xrc, xr_v[:, 0:4, :])
                    xrs_of[0] = xrc
            flush_pv(keep=0)
            pck, pct, ppool = pending
            rcp = emit_denoms(pck, pct, ppool)
            for t in range(len(pck[2])):
                emit_out_tile(pck, t, pct, rcp, psE[t % 2], tail=True)

    nc.compile()
    return nc


def get_program(n: int = N_FULL, h_bias_zero: bool = False):
    key = (n, h_bias_zero)
    if key not in _CACHE:
        _CACHE[key] = _build(n, h_bias_zero)
    return _CACHE[key]


def make_weight_maps(Wf, bf, Wg, bg, Wh, bh, Wv, bv, gamma, h_bias_zero=False):
    """Host-side layout prep of the tiny replicated weights."""
    wv_aug = np.concatenate(
        [np.float32(gamma) * np.asarray(Wv, np.float32),
         np.asarray(bv, np.float32)[None, :]], axis=0)
    bfg = np.concatenate(
        [np.asarray(bf, np.float32), np.asarray(bg, np.float32)]).reshape(P, 1)
    wfg = np.concatenate(
        [np.asarray(Wf, np.float32), np.asarray(Wg, np.float32)], axis=1)
    # c index decomposition: c = cc*128 + p  ->  [p, cc, d]
    maps = {
        "wfg": np.ascontiguousarray(
            wfg.astype(BF16).reshape(CC, P, 2 * D).transpose(1, 0, 2)),
        "wh": np.ascontiguousarray(
            np.asarray(Wh, np.float32).astype(BF16).reshape(CC, P, D).transpose(1, 0, 2)),
        "bfg": np.ascontiguousarray(bfg),
        "bhp": np.ascontiguousarray(
            np.asarray(bh, np.float32).astype(BF16).reshape(1, D)),
        "onesp": np.ones((1, P), dtype=BF16),
        "wv": np.ascontiguousarray(wv_aug.astype(BF16)),
    }
    if h_bias_zero:
        del maps["bhp"], maps["onesp"]
    return maps


def make_x_maps(xf_b):
    """Per-core x layouts: residual rows (bf16) + transposed bf16 [p, cc, n]."""
    x = np.ascontiguousarray(xf_b, dtype=np.float32)
    xt = x.T.astype(BF16).reshape(CC, P, x.shape[0]).transpose(1, 0, 2)
    return {"xr": x.astype(BF16), "xt": np.ascontiguousarray(xt)}


def kernel(x, Wf, bf, Wg, bg, Wh, bh, Wv, bv, gamma):
    from concourse.bass_utils import run_bass_kernel_spmd

    x = np.asarray(x, np.float32)
    b, hh, ww, c = x.shape
    n = hh * ww
    assert (b, c) == (B, C)

    hbz = bool(np.all(np.asarray(bh) == 0))
    nc = get_program(n, hbz)
    base = make_weight_maps(Wf, bf, Wg, bg, Wh, bh, Wv, bv, gamma, hbz)
    xf = x.reshape(b, n, c)
    in_maps = [dict(base, **make_x_maps(xf[i])) for i in range(b)]

    res = run_bass_kernel_spmd(nc, in_maps, core_ids=list(range(b)))
    out = np.stack([np.asarray(res.results[i]["out"], np.float32)
                    for i in range(b)], axis=0)
    return np.ascontiguousarray(out.reshape(b, hh, ww, c).astype(np.float32))


# revision 48
# speedup vs baseline: 1.2446x; 1.0052x over previous
"""SAGAN self-attention block on 8 TRN2 NeuronCores (v7, ~174-176us; v4 was 202us).

Reference (per batch element b, N = H*W = 4096, C = 512, D = 64):
    f = x @ Wf + bf ; g = x @ Wg + bg ; h = x @ Wh + bh      # [N, D]
    s = f @ g.T                                              # [N, N]
    attn = softmax(s, axis=-1)
    ctx = attn @ h                                           # [N, D]
    o = (gamma * ctx) @ Wv + bv + x                          # [N, C]

Sharding: data-parallel over batch B=8 -> one batch element per core, no
collectives. Weights replicated.

Device algorithm (per core), bf16 matmuls with f32 PSUM accumulation:
  - s is computed per m-tile (keys on partitions) in 3-m-tile groups; QK
    pairs row-pack via the FG2/GF2 stacked+mirrored f/g tensors (K=64
    streams 2 cols/cycle with tile_position).
  - softmax is unnormalized (no max subtraction); denominators ride as a
    ones-column in haug through the PV accumulation, are PE-transposed to
    per-partition scalars, and one DVE reciprocal per chunk feeds the
    fused (out*rc + x) epilogue.
  - EXP of the 16.7M logits is the ScalarE bottleneck (0.833ns/elem/lane
    = 109us floor), so the 3rd tile of each triple group is offloaded to
    the DVE as a Schraudolph exp: bf16_bits(e^s) ~= s*(2^7/ln2) + B in
    int16, one tensor_scalar per tile.  gamma=0.01 makes the attention
    term only ~0.8%% of the output norm, so the ~3%% max rel err of the
    approximation costs ~3e-5 end-to-end (gate is 2e-2).
  - PV is software-pipelined two groups behind QK/EXP so the DVE exp hop
    is never on the PE critical path; ctx PSUM ping-pongs between banks
    6/7 per chunk; epilogue (denoms at g==2, out-proj at g 3..6) rides
    inside the next chunk.
  - residual x rows and the output stream are bf16 (12.3MB DMA/core);
    DMA is spread over the two HWDGE queues (Sync: first-chunk slices,
    f/g mirrors, xr slabs; Act: weights + bulk xt slabs) plus GPSIMD
    SWDGE for output stores; final stores split 3 ways across queues.
  - chunk 0 JIT-interleaves the f/g/h projections with its QK groups;
    h accumulates in a PSUM arena in the idle odd ctx bank in rounds of
    8 chains + one CAST (the conservative tile-granular WAR between arena
    writers and CAST readers then bites once per 8 tiles, not per chain).
  - the f/g mirrors ride the GPSIMD SWDGE queue, pre-warmed by the bfg
    load (cold start ~10us), so both HWDGE queues carry xt slabs and all
    of xt lands ~7us earlier.
"""

import numpy as np
import ml_dtypes

BF16 = ml_dtypes.bfloat16

B, HH, WW, C = 8, 64, 64, 512
D = C // 8          # 64
N_FULL = HH * WW    # 4096
P = 128
CC = C // P         # 4  (c-chunks of 128)

_CACHE: dict = {}


def _groups(n_tiles):
    """m-tile groups per n-chunk: triples + a final pair (e.g. 10x3 + 1x2)."""
    gs = []
    i = 0
    while n_tiles - i >= 3:
        if n_tiles - i == 4:
            break
        gs.append([i, i + 1, i + 2])
        i += 3
    while i < n_tiles:
        gs.append(list(range(i, min(i + 2, n_tiles))))
        i += 2
    return gs


def _build(n: int, h_bias_zero: bool = False):
    import concourse.mybir as mybir
    from concourse import bacc
    from concourse.tile import TileContext

    f32 = mybir.dt.float32
    bf16 = mybir.dt.bfloat16
    i16 = mybir.dt.int16
    # Schraudolph exp in bf16-bit space: bf16_bits(exp(s)) ~= s*(2^7/ln2) + B.
    # One DVE tensor_scalar (f32 PSUM -> int16 SBUF) computes a ~3% max-rel-err
    # exp; with gamma=0.01 scaling the attention term, the end-to-end error is
    # ~3e-5.  Used for the 3rd tile of each triple group outside chunk 0 to
    # offload ~1/3 of the softmax EXP stream from the ScalarE bottleneck.
    EXP_A = float(128.0 / np.log(2.0))
    EXP_B = 16250.625
    ADD = mybir.AluOpType.add
    MULT = mybir.AluOpType.mult
    EXP = mybir.ActivationFunctionType.Exp

    n_tiles = n // P        # 32
    nch = n // 512          # 8

    nc = bacc.Bacc("TRN2", target_bir_lowering=False, debug=False)

    xr_d = nc.dram_tensor("xr", [n, C], bf16, kind="ExternalInput")
    xt_d = nc.dram_tensor("xt", [P, CC, n], bf16, kind="ExternalInput")
    wfg_d = nc.dram_tensor("wfg", [P, CC, 2 * D], bf16, kind="ExternalInput")
    wh_d = nc.dram_tensor("wh", [P, CC, D], bf16, kind="ExternalInput")
    bfg_d = nc.dram_tensor("bfg", [P, 1], f32, kind="ExternalInput")   # [bf;bg]
    if not h_bias_zero:
        bh_d = nc.dram_tensor("bhp", [1, D], bf16, kind="ExternalInput")
        on_d = nc.dram_tensor("onesp", [1, P], bf16, kind="ExternalInput")
    wv_d = nc.dram_tensor("wv", [D + 1, C], bf16, kind="ExternalInput")
    out_d = nc.dram_tensor("out", [n, C], bf16, kind="ExternalOutput")

    xr_v = xr_d.rearrange("(i p) c -> p i c", p=P)
    o_t = out_d.rearrange("(i p) c -> i p c", p=P)

    groups = _groups(n_tiles)
    need_fg = [grp[-1] // 4 for grp in groups]

    with TileContext(nc) as tc:
        with (
            tc.tile_pool(name="const", bufs=1) as cpool,
            tc.tile_pool(name="big", bufs=1) as bigpool,
            tc.tile_pool(name="ep", bufs=5) as epool,
            tc.tile_pool(name="eps", bufs=5) as epspool,
            tc.tile_pool(name="ct", bufs=2) as ctpool,
            tc.tile_pool(name="os", bufs=4) as opool,
            tc.tile_pool(name="xr", bufs=8) as xrpool,
            tc.tile_pool(name="sm", bufs=4) as smpool,
            tc.tile_pool(name="psSP", bufs=2, space="PSUM") as psSP,
            tc.tile_pool(name="psS3", bufs=2, space="PSUM") as psS3,
            tc.tile_pool(name="psE0", bufs=1, space="PSUM") as psE0,
            tc.tile_pool(name="psE1", bufs=1, space="PSUM") as psE1,
        ):
            psE = [psE0, psE1]

            # ---- replicated constants -> SBUF (small ones on qAct)
            # bfg rides the GPSIMD SWDGE queue first to warm it up: the f/g
            # mirrors use it mid-prologue and its cold start is ~10us.
            bfg_sb = cpool.tile([P, 1], f32)
            nc.gpsimd.dma_start(bfg_sb, bfg_d[:, :])
            wfg_sb = cpool.tile([P, CC, 2 * D], bf16)
            nc.scalar.dma_start(wfg_sb, wfg_d[:, :, :])

            # ---- persistent SBUF tensors
            xt = bigpool.tile([P, CC, n], bf16)          # x.T (c on partitions)
            FG2 = bigpool.tile([P, n], bf16)             # rows 0:64 f.T, 64:128 g.T
            GF2 = bigpool.tile([P, n], bf16)             # rows 0:64 g.T, 64:128 f.T
            haug = bigpool.tile([P, n_tiles, D + 1], bf16)
            nc.gpsimd.memset(haug[:, :, D:D + 1], 1.0)

            # xt DMAs: chunk 0+1 split across both HW queues (per-cc slices
            # for fast first-chunk), later chunks as one batched slab each so
            # the Sync sequencer only spends ~0.6us per chunk.
            for jc in range(2):
                for cc in range(CC):
                    sl = slice(jc * 512, (jc + 1) * 512)
                    eng = nc.sync if (cc % 2 == 0) else nc.scalar
                    eng.dma_start(xt[:, cc, sl], xt_d[:, cc, sl])

            wh_sb = cpool.tile([P, CC, D], bf16)
            nc.scalar.dma_start(wh_sb, wh_d[:, :, :])
            if not h_bias_zero:
                bh_sb = cpool.tile([1, D], bf16)
                nc.scalar.dma_start(bh_sb, bh_d[:, :])
                ones_sb = cpool.tile([1, P], bf16)
                nc.scalar.dma_start(ones_sb, on_d[:, :])
            wv_sb = cpool.tile([D + 1, C], bf16)
            nc.scalar.dma_start(wv_sb, wv_d[:, :])

            # bulk xt slabs split across BOTH HWDGE queues (mirrors ride the
            # pre-warmed SWDGE queue, so neither HW queue must stay empty);
            # all of xt lands ~7us earlier, unblocking the tail fg chunks.
            # qSyIo gets only slab-2 so the latency-critical mirror-0/1
            # transfers are never behind bulk xt in its FIFO (the SWDGE
            # queue turned out to be ~4us push-to-data + ~1.5us/transfer,
            # which made QK-g0 wait until ~24.7us)
            for jc in range(2, nch):
                sl = slice(jc * 512, (jc + 1) * 512)
                eng = nc.sync if jc == 2 else nc.scalar
                eng.dma_start(xt[:, :, sl], xt_d[:, :, sl])

            def emit_fg_chunk(jc):
                """f/g projection for 512-chunk jc -> FG2 + GF2 mirror."""
                sl = slice(jc * 512, (jc + 1) * 512)
                # fg chunks 2+ rotate through psS3, whose waits are all
                # DVE-released (schraud/bias), never ACT -- this breaks the
                # chunk-0 PE->ACT->PE stutter ring in the sp pool
                pool, tag = (psSP, "sp") if jc < 2 else (psS3, "s3")
                fgp = pool.tile([P, 512], f32, tag=tag, name=f"fg{jc}")
                for cc in range(CC):
                    nc.tensor.matmul(
                        fgp, lhsT=wfg_sb[:, cc, :], rhs=xt[:, cc, sl],
                        start=(cc == 0), stop=(cc == CC - 1),
                    )
                nc.vector.tensor_scalar(FG2[:, sl], fgp, bfg_sb, None, ADD)
                # mirror halves swapped (latency-critical: QK g0 needs them)
                nc.sync.dma_start(GF2[D:P, sl], FG2[0:D, sl])
                nc.sync.dma_start(GF2[0:D, sl], FG2[D:P, sl])

            # h arena: 8 rotating [128, 64] slots in PSUM bank 7 (psE1's bank
            # is otherwise unused until ctx_1 at chunk 1).
            hparena = psE1.tile([P, 512], f32, tag="cx", name="hparena")

            def emit_h_oct(r):
                """h projection for m-tiles 8r..8r+7: 8 accumulation chains
                into the arena, then ONE CAST into haug.  All writes precede
                the single read, so the conservative tile-granular WAR between
                arena writers and haug-CAST readers bites once per 8 tiles
                instead of once per chain."""
                for t in range(8):
                    i = 8 * r + t
                    hp = hparena[:, t * D:(t + 1) * D]
                    for cc in range(CC):
                        nc.tensor.matmul(
                            hp, lhsT=xt[:, cc, i * P:(i + 1) * P],
                            rhs=wh_sb[:, cc, :],
                            start=(cc == 0), stop=(h_bias_zero and cc == CC - 1),
                        )
                    if not h_bias_zero:
                        nc.tensor.matmul(
                            hp, lhsT=ones_sb, rhs=bh_sb, start=False, stop=True)
                # CAST on the ScalarE: it is idle until its first EXP, while
                # the DVE carries the bias->mirror chain that gates QK
                nc.scalar.copy(
                    haug[:, 8 * r:8 * r + 8, 0:D],
                    hparena.rearrange("p (t d) -> p t d", d=D))

            def emit_qk_exp(ck, g, offload=False):
                """QK + EXP for m-tile group g of column-chunk ck.
                Tiles 0-1 go to the 2-bank sp pool (released by the ScalarE
                EXP alone); tile 2 goes to its own 1-bank pool (released by
                the DVE Schraudolph alone), so a lag on either exp engine
                no longer stalls BOTH QK allocations two groups later.
                Returns (ep, ep2, ep2_is_i16)."""
                cs, cw, _ = ck
                sl = slice(cs, cs + cw)
                grp = groups[g]

                def qk(dst, q, i):
                    # row-pack QK by m-tile parity: even tiles use rows 0:64
                    # (g in GF2, f in FG2), odd tiles rows 64:128.
                    hb = (i % 2) * D
                    lhs = GF2 if hb == 0 else FG2
                    rhs = FG2 if hb == 0 else GF2
                    nc.tensor.matmul(
                        dst[:, q * 512:q * 512 + cw],
                        lhsT=lhs[hb:hb + D, i * P:(i + 1) * P],
                        rhs=rhs[hb:hb + D, sl],
                        start=True, stop=True, tile_position=(hb, 0),
                    )

                sp = psSP.tile([P, 1024], f32, tag="sp", name=f"sp{cs}_{g}")
                qk(sp, 0, grp[0])
                if len(grp) > 1:
                    qk(sp, 1, grp[1])
                sp3 = None
                if len(grp) == 3:
                    sp3 = psS3.tile([P, 512], f32, tag="s3", name=f"s3{cs}_{g}")
                    qk(sp3, 0, grp[2])
                ep = epool.tile([P, 1024], bf16, tag="ep")
                spv = sp.rearrange("p (q v) -> p q v", v=512)
                epv = ep.rearrange("p (q v) -> p q v", v=512)
                nw = min(2, len(grp))
                nc.scalar.activation(epv[:, 0:nw, 0:cw], spv[:, 0:nw, 0:cw], EXP)
                if sp3 is None:
                    return ep, None, False
                if offload:
                    epS = epspool.tile([P, 512], i16, tag="eps")
                    nc.vector.tensor_scalar(
                        epS[:, 0:cw], sp3[:, 0:cw], EXP_A, EXP_B, MULT, ADD)
                    return ep, epS, True
                ep3 = epspool.tile([P, 512], bf16, tag="eps")
                nc.scalar.activation(ep3[:, 0:cw], sp3[:, 0:cw], EXP)
                return ep, ep3, False

            def emit_pv(ck, g, ctx, eps):
                ep, ep2, is_i16 = eps
                cs, cw, _ = ck
                for q, i in enumerate(groups[g]):
                    if q == 2:
                        rhs = (ep2.bitcast(bf16) if is_i16 else ep2)[:, 0:cw]
                    else:
                        rhs = ep[:, q * 512:q * 512 + cw]
                    nc.tensor.matmul(
                        ctx[:, 0:cw], lhsT=haug[:, i, :], rhs=rhs,
                        start=(g == 0 and q == 0), stop=(i == n_tiles - 1),
                    )

            def emit_ct_copy(ck, ctx):
                """ctx -> bf16 SBUF copy (DVE)."""
                cs, cw, _ = ck
                ct = ctpool.tile([D + 1, 512], bf16, tag="ct", name=f"ct{cs}")
                nc.vector.tensor_copy(out=ct[:, 0:cw], in_=ctx[:, 0:cw])
                return ct

            def emit_denoms(ck, ct, pool):
                """PE transposes of the denominator row into one PSUM tile in
                the pending chunk's ping-pong bank, then one DVE reciprocal."""
                _, cw, tiles = ck
                # bf16 PSUM writes need 4-byte alignment: space columns 2 apart
                dt4 = pool.tile([P, 8], bf16, tag="cx", name=f"dt{tiles[0]}")
                for t in range(len(tiles)):
                    tsl = slice(t * P, (t + 1) * P)
                    nc.tensor.transpose(
                        dt4[:, 2 * t:2 * t + 1], ct[D:D + 1, tsl],
                        haug[D:D + 1, 0, D:D + 1])
                rc4 = smpool.tile([P, 8], f32, tag="rc")
                nc.vector.reciprocal(rc4, dt4)
                return rc4

            def emit_out_tile(ck, t, ct, rc, pool, tail=False):
                """out-proj + scale + residual + store for one 128-row tile."""
                it = ck[2][t]
                tsl = slice(t * P, (t + 1) * P)
                op = pool.tile([P, C], f32, tag="cx", name=f"op{it}")
                nc.tensor.matmul(op, lhsT=ct[:, tsl], rhs=wv_sb, start=True, stop=True)
                osb = opool.tile([P, C], bf16, tag="os")
                nc.vector.scalar_tensor_tensor(
                    out=osb, in0=op, scalar=rc[:, 2 * t:2 * t + 1],
                    in1=xrs_of[it // 4][:, it % 4, :], op0=MULT, op1=ADD)
                if tail:
                    # final stores: split across SWDGE + Sync + Act queues so
                    # the drain after the last compute is short
                    nc.gpsimd.dma_start(o_t[it][0:48, :], osb[0:48, :])
                    nc.sync.dma_start(o_t[it][48:96, :], osb[48:96, :])
                    nc.scalar.dma_start(o_t[it][96:P, :], osb[96:P, :])
                else:
                    nc.gpsimd.dma_start(o_t[it], osb)

            # ---- emission schedule -------------------------------------
            chunks = [(j * 512, 512, [4 * j + t for t in range(4)])
                      for j in range(nch)]

            fg_done = 0
            h_done = 0
            xrs_of = {}
            pending = None   # epilogue state: (ck, ct, pool)
            pv_q = []        # deferred PVs: (ck, g, ctx, eps), depth 2
            rcp = None

            def flush_pv(keep=2):
                # PV runs two groups behind its QK/EXP so the DVE-offloaded
                # exp tile is never on the PV critical path
                nonlocal pending
                while len(pv_q) > keep:
                    pck0, g0, ctx0, ep0 = pv_q.pop(0)
                    emit_pv(pck0, g0, ctx0, ep0)
                    if g0 == len(groups) - 1:
                        # chunk-final PV: ctx done -> bf16 copy, open epilogue
                        pending = (pck0, emit_ct_copy(pck0, ctx0),
                                   psE[(pck0[0] // 512) % 2])

            for ci, ck in enumerate(chunks):
                cs, cw, tiles = ck
                first = (ci == 0)
                if not first:
                    # residual rows (consumed by this chunk's epilogue during
                    # the next chunk); chunk 0's slab is deferred so it does
                    # not delay the latency-critical f/g mirrors on qSyIo.
                    xrc = xrpool.tile([P, 4, C], bf16, tag="xr", name=f"xr{ci}")
                    nc.sync.dma_start(xrc, xr_v[:, ci * 4:(ci + 1) * 4, :])
                    xrs_of[ci] = xrc
                ctx = psE[ci % 2].tile([D + 1, 512], f32, tag="cx", name=f"ctx{cs}")
                for g, grp in enumerate(groups):
                    if first:
                        # fg rides one chunk ahead of QK demand; h pairs are
                        # emitted AFTER the group's QK/EXP so the first EXPs
                        # are never queued behind them on the in-order PE.
                        # two fg chunks per group boundary: consecutive fg
                        # allocs in the sp pool then wait each other's fast
                        # DVE bias reads instead of EXPs, and fg7's chains
                        # issue right as its xt slab lands (~19us)
                        while fg_done <= min(2 * g + 1, nch - 1):
                            emit_fg_chunk(fg_done)
                            fg_done += 1
                    ep = emit_qk_exp(ck, g, offload=not first)
                    pv_q.append((ck, g, ctx, ep))
                    flush_pv()
                    if first:
                        while (8 * h_done < 4 * fg_done
                               and 8 * h_done <= grp[-1] + 8):
                            emit_h_oct(h_done)
                            h_done += 1
                    if pending is not None:
                        pck, pct, ppool = pending
                        if g == 2:
                            rcp = emit_denoms(pck, pct, ppool)
                        elif 3 <= g <= len(pck[2]) + 2:
                            emit_out_tile(pck, g - 3, pct, rcp, ppool)
                            if g == len(pck[2]) + 2:
                                pending = None
                if first:
                    xrc = xrpool.tile([P, 4, C], bf16, tag="xr", name="xr0")
                    nc.sync.dma_start(xrc, xr_v[:, 0:4, :])
                    xrs_of[0] = xrc
            flush_pv(keep=0)
            pck, pct, ppool = pending
            rcp = emit_denoms(pck, pct, ppool)
            for t in range(len(pck[2])):
                emit_out_tile(pck, t, pct, rcp, psE[t % 2], tail=True)

    nc.compile()
    return nc


def get_program(n: int = N_FULL, h_bias_zero: bool = False):
    key = (n, h_bias_zero)
    if key not in _CACHE:
        _CACHE[key] = _build(n, h_bias_zero)
    return _CACHE[key]


def make_weight_maps(Wf, bf, Wg, bg, Wh, bh, Wv, bv, gamma, h_bias_zero=False):
    """Host-side layout prep of the tiny replicated weights."""
    wv_aug = np.concatenate(
        [np.float32(gamma) * np.asarray(Wv, np.float32),
         np.asarray(bv, np.float32)[None, :]], axis=0)
    bfg = np.concatenate(
        [np.asarray(bf, np.float32), np.asarray(bg, np.float32)]).reshape(P, 1)
    wfg = np.concatenate(
        [np.asarray(Wf, np.float32), np.asarray(Wg, np.float32)], axis=1)
    # c index decomposition: c = cc*128 + p  ->  [p, cc, d]
    maps = {
        "wfg": np.ascontiguousarray(
            wfg.astype(BF16).reshape(CC, P, 2 * D).transpose(1, 0, 2)),
        "wh": np.ascontiguousarray(
            np.asarray(Wh, np.float32).astype(BF16).reshape(CC, P, D).transpose(1, 0, 2)),
        "bfg": np.ascontiguousarray(bfg),
        "bhp": np.ascontiguousarray(
            np.asarray(bh, np.float32).astype(BF16).reshape(1, D)),
        "onesp": np.ones((1, P), dtype=BF16),
        "wv": np.ascontiguousarray(wv_aug.astype(BF16)),
    }
    if h_bias_zero:
        del maps["bhp"], maps["onesp"]
    return maps


def make_x_maps(xf_b):
    """Per-core x layouts: residual rows (bf16) + transposed bf16 [p, cc, n]."""
    x = np.ascontiguousarray(xf_b, dtype=np.float32)
    xt = x.T.astype(BF16).reshape(CC, P, x.shape[0]).transpose(1, 0, 2)
    return {"xr": x.astype(BF16), "xt": np.ascontiguousarray(xt)}


def kernel(x, Wf, bf, Wg, bg, Wh, bh, Wv, bv, gamma):
    from concourse.bass_utils import run_bass_kernel_spmd

    x = np.asarray(x, np.float32)
    b, hh, ww, c = x.shape
    n = hh * ww
    assert (b, c) == (B, C)

    hbz = bool(np.all(np.asarray(bh) == 0))
    nc = get_program(n, hbz)
    base = make_weight_maps(Wf, bf, Wg, bg, Wh, bh, Wv, bv, gamma, hbz)
    xf = x.reshape(b, n, c)
    in_maps = [dict(base, **make_x_maps(xf[i])) for i in range(b)]

    res = run_bass_kernel_spmd(nc, in_maps, core_ids=list(range(b)))
    out = np.stack([np.asarray(res.results[i]["out"], np.float32)
                    for i in range(b)], axis=0)
    return np.ascontiguousarray(out.reshape(b, hh, ww, c).astype(np.float32))
